# revision 13
# baseline (speedup 1.0000x reference)
"""DeepONet+GRU Trainium2 kernel (8-core data parallel), v3.

Full inputs in, full outputs out. Batch 1024 is sharded 128/core across 8
NeuronCores; all parameters are replicated. Per core:
  branch MLP (528->512 x4, fp16, transposed layout) -> branchT [512f, 128b]
  trunk MLP (1->512 x4, fp16) -> trunkT [512f, 128t]
  seq = branchT.T @ trunkT (+don_bias folded into GRU aug weights)
  2-layer GRU scan over T=128 steps, hidden 256, fully on-chip
  proj -> [128, 16]

v3 GRU structure:
  - all gate biases + input-side terms folded into PSUM via K=1/K=2
    matmuls (st = [seq_t; ones] rank-2 trick extended to the n-gate)
  - fp16 state + gates throughout
  - layer-0 gate chain split into GH-halves (wavefront): the lo-half of
    h0T is copied out while the hi-half chain still runs, so the next
    step's first recurrent matmuls start earlier. whh0/aug0 columns are
    host-permuted to (r_lo z_lo | r_hi z_hi | n_lo n_hi).
  - (z-1)*n fused via scalar_tensor_tensor; h = z*h - (z-1)*n
  - engine balance: ACT does sigmoids/tanhs + h1T copy, DVE does the
    mul/add chain + h0T half copies, Pool does z*h products and (z-1)*n
    for layer 0
  - low-priority junk filler matmuls let the scheduler keep the PE
    p-state ramped (PE runs 0.83ns/col after any idle gap, 0.417ns/col
    when continuously busy >3us)
"""
import sys
sys.path.insert(0, '/opt/trn_rl_repo')

import numpy as np

B = 1024
BC = 128          # batch per core
NB = 528
NBP = 640         # padded branch input (5 k-tiles)
HID = 512
GH = 256
T = 128
NS = 16
NCORES = 8

N_FILL = 6        # junk 256-col matmuls per step (PE p-state keep-alive)

_CACHE = {}


def _patched_tile_context(nc):
    """TileContext whose tail drain splits sem waits (walrus CoreV3 rejects
    >1 sync wait on a Drain)."""
    import concourse.tile as tile
    from concourse.vector_clock import ScopedClock

    class PatchedTileContext(tile.TileContext):
        def _drain_and_barrier(self, tick_clock, wait_clock):
            nc = self.nc
            drain_inst = nc.sync.drain()
            wait_clock.add_sem_waits(
                drain_inst.ins, ScopedClock({None: tick_clock.global_clock})
            )
            si = drain_inst.ins.sync_info
            waits = list(si.on_wait or []) if si is not None else []
            if len(waits) > 1:
                si.on_wait = waits[:1]
                for i in range(1, len(waits)):
                    extra = nc.sync.drain()
                    esi = extra.ins.sync_info
                    if esi is None:
                        from concourse import mybir
                        extra.ins.sync_info = mybir.SyncInfo(
                            on_wait=waits[i:i + 1], on_update=[]
                        )
                    else:
                        esi.on_wait = waits[i:i + 1]
            nc.all_engine_barrier()
            assert self.sems is not None
            popped = nc._tile_sem_poison_stack.pop()
            assert popped is self._sem_poison
            nc.clear_and_free_semaphores(list(self.sems.allocated().values()))
            nc.all_engine_barrier()

    return PatchedTileContext(nc)


def _split_multi_waits(nc):
    """This container's walrus rejects >1 sync wait per instruction
    ("Too many sync wait commands"). Hoist extra waits onto engine-matched
    NoOps spliced immediately before the offending instruction."""
    from concourse import mybir
    n_extra = 0
    for fn in nc.m.functions:
        for bb in fn.blocks:
            new = []
            for inst in bb.instructions:
                si = inst.sync_info
                waits = list(si.on_wait) if (si is not None and si.on_wait) else []
                if len(waits) > 1:
                    for w in waits[:-1]:
                        nop = mybir.InstNoOp(
                            name=f"wsplit-{n_extra}-{inst.name}",
                            engine=inst.engine,
                            bass_nofuse=True,
                            sync_info=mybir.SyncInfo(on_wait=[w], on_update=[]),
                        )
                        new.append(nop)
                        n_extra += 1
                    si.on_wait = [waits[-1]]
                new.append(inst)
            if n_extra:
                bb.instructions[:] = new
    return n_extra


def build_nc(n_steps=T):
    import concourse.bass as bass
    from concourse import mybir
    from contextlib import ExitStack

    FP = mybir.dt.float32
    HF = mybir.dt.float16
    AF = mybir.ActivationFunctionType
    ALU = mybir.AluOpType
    nc = bass.Bass()

    def mm(out, lhsT, rhs, start, stop):
        nc.tensor.matmul(out, lhsT, rhs, start=start, stop=stop)

    # ---- DRAM parameters (host-prepped layouts) ----
    dp = lambda name, shape, dt=FP: nc.declare_dram_parameter(name, list(shape), dt, isOutput=False)
    xT_d = dp("xT", (5, 128, BC), HF)
    bW_d = [dp("bW0", (5, 128, HID), HF)] + [dp(f"bW{i}", (4, 128, HID), HF) for i in (1, 2, 3)]
    bb_d = [dp(f"bb{i}", (128, 4)) for i in range(4)]
    tW0_d = dp("tW0", (1, HID), HF)
    tW_d = [None] + [dp(f"tW{i}", (4, 128, HID), HF) for i in (1, 2, 3)]
    tb_d = [dp(f"tb{i}", (128, 4)) for i in range(4)]
    tT_d = dp("tT", (1, T), HF)
    whh0_d = dp("whh0", (2, 128, 3 * GH), HF)
    whh1_d = dp("whh1", (2, 128, 3 * GH), HF)
    wih1_d = dp("wih1", (2, 128, 3 * GH), HF)
    aug0_d = dp("aug0", (2, 768), HF)
    aug0n_d = dp("aug0n", (2, 256), HF)
    aug1_d = dp("aug1", (1, 768), HF)
    aug1n_d = dp("aug1n", (1, 256), HF)
    pW_d = dp("pW", (2, 128, NS), HF)
    pb_d = dp("pb", (1, NS), HF)
    ident16_d = dp("ident16", (128, 128), HF)
    zer_d = dp("zer", (128, GH), HF)
    ones16k_d = dp("ones16k", (1, T * BC), HF)
    out_d = nc.declare_dram_parameter("out", [BC, NS], FP, isOutput=True)

    with ExitStack() as ctx:
        tc = ctx.enter_context(_patched_tile_context(nc))
        const = ctx.enter_context(tc.tile_pool(name="const", bufs=1))

        # ---- persistent SBUF ----
        ident16 = const.tile([128, 128], HF)
        nc.gpsimd.dma_start(ident16[:], ident16_d[:])
        whh0 = const.tile([128, 2 * 768], HF)
        whh1 = const.tile([128, 2 * 768], HF)
        wih1 = const.tile([128, 2 * 768], HF)
        for k in range(2):
            nc.gpsimd.dma_start(whh0[:, k * 768:(k + 1) * 768], whh0_d[k])
            nc.gpsimd.dma_start(whh1[:, k * 768:(k + 1) * 768], whh1_d[k])
            nc.gpsimd.dma_start(wih1[:, k * 768:(k + 1) * 768], wih1_d[k])
        aug0 = const.tile([2, 768], HF)
        nc.gpsimd.dma_start(aug0[:], aug0_d[:])
        aug0n = const.tile([2, 256], HF)
        nc.gpsimd.dma_start(aug0n[:], aug0n_d[:])
        aug1 = const.tile([1, 768], HF)
        nc.gpsimd.dma_start(aug1[:], aug1_d[:])
        aug1n = const.tile([1, 256], HF)
        nc.gpsimd.dma_start(aug1n[:], aug1n_d[:])
        pW = const.tile([128, 2 * NS], HF)
        for k in range(2):
            nc.gpsimd.dma_start(pW[:, k * NS:(k + 1) * NS], pW_d[k])
        pb = const.tile([1, NS], HF)
        nc.gpsimd.dma_start(pb[:], pb_d[:])
        ones1 = const.tile([1, 128], HF)
        nc.gpsimd.dma_start(ones1[:], ones16k_d[:, 0:128])

        branchT = const.tile([128, HID], HF)  # [feat within tile, 4 mtiles * batch]
        trunkT = const.tile([128, HID], HF)
        seqT_sb = const.tile([T, BC], HF)
        seq16 = const.tile([BC, T], HF)    # [batch, t] fp16 (transpose input)

        # states (fp16): h in [batch, GH]; hT in [GH-halves, batch]
        h0 = const.tile([128, GH], HF)
        h1 = const.tile([128, GH], HF)
        h0T = const.tile([128, GH], HF)
        h1T = const.tile([128, GH], HF)
        nc.vector.memset(h0[:], 0.0)
        nc.vector.memset(h1[:], 0.0)
        nc.gpsimd.dma_start(h0T[:], zer_d[:])
        nc.gpsimd.dma_start(h1T[:], zer_d[:])

        # ================= MLP phase (fp16) =================
        with tc.tile_pool(name="mlpw", bufs=1) as mlpw, \
             tc.tile_pool(name="mlps", bufs=2) as mlps, \
             tc.tile_pool(name="mlpp", bufs=4, space=bass.MemorySpace.PSUM) as mlpp:

            def mlp(xtiles_sb, nk_first, W_sbs, b_sbs, final_relu, out_sb):
                cur = xtiles_sb
                nlayers = 4
                for l in range(nlayers):
                    nk = nk_first if l == 0 else 4
                    Wl = W_sbs[l]
                    dst = out_sb if l == nlayers - 1 else mlps.tile([128, HID], HF, tag="mlpact")
                    for m in range(4):
                        ps = mlpp.tile([128, 128], FP, tag="mlppsum")
                        for k in range(nk):
                            mm(
                                ps[:],
                                Wl[:, k * HID + m * 128: k * HID + (m + 1) * 128],
                                cur[:, k * 128:(k + 1) * 128],
                                start=(k == 0), stop=(k == nk - 1),
                            )
                        func = AF.Relu if (l < nlayers - 1 or final_relu) else AF.Identity
                        nc.scalar.activation(
                            dst[:, m * 128:(m + 1) * 128], ps[:], func,
                            bias=b_sbs[l][:, m:m + 1],
                        )
                    cur = dst
                return cur

            # branch weights -> SBUF
            bW_sb = []
            for l in range(4):
                nk = 5 if l == 0 else 4
                w = mlpw.tile([128, nk * HID], HF, tag=f"bw{l}")
                for k in range(nk):
                    nc.gpsimd.dma_start(w[:, k * HID:(k + 1) * HID], bW_d[l][k])
                bW_sb.append(w)
            bb_sb = []
            for l in range(4):
                t_ = mlpw.tile([128, 4], FP, tag=f"bb{l}")
                nc.gpsimd.dma_start(t_[:], bb_d[l][:])
                bb_sb.append(t_)
            xk = mlpw.tile([128, 5 * 128], HF, tag="xk")
            for k in range(5):
                nc.gpsimd.dma_start(xk[:, k * 128:(k + 1) * 128], xT_d[k])
            mlp(xk, 5, bW_sb, bb_sb, final_relu=False, out_sb=branchT)

            # trunk: first layer K=1
            tW0 = mlpw.tile([1, HID], HF, tag="tw0")
            nc.gpsimd.dma_start(tW0[:], tW0_d[:])
            tTs = mlpw.tile([1, T], HF, tag="tts")
            nc.gpsimd.dma_start(tTs[:], tT_d[:])
            tb_sb = []
            for l in range(4):
                t_ = mlpw.tile([128, 4], FP, tag=f"tb{l}")
                nc.gpsimd.dma_start(t_[:], tb_d[l][:])
                tb_sb.append(t_)
            tW_sb = [None]
            for l in (1, 2, 3):
                w = mlpw.tile([128, 4 * HID], HF, tag=f"tw{l}")
                for k in range(4):
                    nc.gpsimd.dma_start(w[:, k * HID:(k + 1) * HID], tW_d[l][k])
                tW_sb.append(w)

            tact = mlps.tile([128, HID], HF, tag="mlpact")
            for m in range(4):
                ps = mlpp.tile([128, 128], FP, tag="mlppsum")
                mm(ps[:], tW0[:, m * 128:(m + 1) * 128], tTs[:],
                                 start=True, stop=True)
                nc.scalar.activation(tact[:, m * 128:(m + 1) * 128], ps[:],
                                     AF.Relu, bias=tb_sb[0][:, m:m + 1])
            # layers 1..3 of trunk
            cur = tact
            for l in (1, 2, 3):
                dst = trunkT if l == 3 else mlps.tile([128, HID], HF, tag="mlpact")
                for m in range(4):
                    ps = mlpp.tile([128, 128], FP, tag="mlppsum")
                    for k in range(4):
                        mm(
                            ps[:],
                            tW_sb[l][:, k * HID + m * 128: k * HID + (m + 1) * 128],
                            cur[:, k * 128:(k + 1) * 128],
                            start=(k == 0), stop=(k == 3),
                        )
                    nc.scalar.activation(dst[:, m * 128:(m + 1) * 128], ps[:],
                                         AF.Relu, bias=tb_sb[l][:, m:m + 1])
                cur = dst

            # seq[b,t] = sum_f branchT[f,b] * trunkT[f,t]  -> [B, T] psum
            ps_seq = mlpp.tile([128, 128], FP, tag="mlppsum")
            for k in range(4):
                mm(ps_seq[:], branchT[:, k * 128:(k + 1) * 128],
                                 trunkT[:, k * 128:(k + 1) * 128],
                                 start=(k == 0), stop=(k == 3))
            nc.scalar.copy(seq16[:], ps_seq[:])
            ps_seqT = mlpp.tile([128, 128], HF, tag="mlppsumT")
            nc.tensor.transpose(ps_seqT[:], seq16[:], ident16[:])
            nc.scalar.copy(seqT_sb[:], ps_seqT[:])

        # ================= GRU phase =================
        saug = const.tile([2, T * BC], HF)
        # partition-collapse seqT (t-major rows) into row 0 of saug
        nc.gpsimd.dma_start(saug[0:1, :], seqT_sb[:])
        nc.gpsimd.dma_start(saug[1:2, :], ones16k_d[:])

        with tc.tile_pool(name="gp", bufs=1, space=bass.MemorySpace.PSUM) as gp, \
             tc.tile_pool(name="gs", bufs=2) as gs:

            Pfill = gp.tile([128, 256], FP, tag="Pfill")

            def fillers():
                with tc.high_priority(offset=-10**6):
                    for _ in range(N_FILL):
                        mm(Pfill[:], ident16[:], whh0[:, 0:256],
                           start=True, stop=True)

            def l0_mms(t):
                # whh0/aug0 columns host-permuted: (r_lo z_lo | r_hi z_hi | n_lo n_hi)
                st = saug[:, t * BC:(t + 1) * BC]
                Plo = gp.tile([128, 256], FP, tag="Plo")    # (r_lo z_lo)
                mm(Plo[:], st, aug0[:, 0:256], start=True, stop=False)
                mm(Plo[:], h0T[:, 0:128], whh0[:, 0:256], start=False, stop=False)
                mm(Plo[:], h0T[:, 128:256], whh0[:, 768:1024], start=False, stop=True)
                Phi = gp.tile([128, 256], FP, tag="Phi")    # (r_hi z_hi)
                mm(Phi[:], st, aug0[:, 256:512], start=True, stop=False)
                mm(Phi[:], h0T[:, 0:128], whh0[:, 256:512], start=False, stop=False)
                mm(Phi[:], h0T[:, 128:256], whh0[:, 1024:1280], start=False, stop=True)
                AB0 = gp.tile([128, 512], FP, tag="AB0")    # (Bn_lo Bn_hi | A_lo A_hi)
                mm(AB0[:, 0:256], st, aug0[:, 512:768], start=True, stop=False)
                mm(AB0[:, 0:256], h0T[:, 0:128], whh0[:, 512:768], start=False, stop=False)
                mm(AB0[:, 0:256], h0T[:, 128:256], whh0[:, 1280:1536], start=False, stop=True)
                mm(AB0[:, 256:512], st, aug0n[:], start=True, stop=True)
                return Plo, Phi, AB0

            def l1_mms():
                # L1 for step t-1: h0T/h1T hold step t-1 / t-2 states here
                Prz1 = gp.tile([128, 512], FP, tag="Prz1")
                mm(Prz1[:], ones1[:], aug1[:, 0:512], start=True, stop=False)
                mm(Prz1[:], h1T[:, 0:128], whh1[:, 0:512], start=False, stop=False)
                mm(Prz1[:], h1T[:, 128:256], whh1[:, 768:1280], start=False, stop=False)
                mm(Prz1[:], h0T[:, 0:128], wih1[:, 0:512], start=False, stop=False)
                mm(Prz1[:], h0T[:, 128:256], wih1[:, 768:1280], start=False, stop=True)
                AB1 = gp.tile([128, 512], FP, tag="AB1")  # (B1n | A1n)
                mm(AB1[:, 0:256], ones1[:], aug1[:, 512:768], start=True, stop=False)
                mm(AB1[:, 0:256], h1T[:, 0:128], whh1[:, 512:768], start=False, stop=False)
                mm(AB1[:, 0:256], h1T[:, 128:256], whh1[:, 1280:1536], start=False, stop=True)
                mm(AB1[:, 256:512], ones1[:], aug1n[:], start=True, stop=False)
                mm(AB1[:, 256:512], h0T[:, 0:128], wih1[:, 512:768], start=False, stop=False)
                mm(AB1[:, 256:512], h0T[:, 128:256], wih1[:, 1280:1536], start=False, stop=True)
                return Prz1, AB1

            def l0_half(P, AB0, Ptr0, lo):
                # one GH-half of the layer-0 gate chain + transpose + copy
                o = 0 if lo else 128
                rz = gs.tile([128, 256], HF, tag=f"rz0{o}")
                nc.scalar.activation(rz[:], P[:], AF.Sigmoid)   # (r | z)
                zh = gs.tile([128, 128], HF, tag=f"zh0{o}")
                nc.gpsimd.tensor_mul(zh[:], rz[:, 128:256], h0[:, o:o + 128])
                t1 = gs.tile([128, 128], HF, tag=f"t10{o}")
                nc.vector.tensor_mul(t1[:], rz[:, 0:128], AB0[:, o:o + 128])
                t2 = gs.tile([128, 128], HF, tag=f"t20{o}")
                nc.vector.tensor_add(t2[:], t1[:], AB0[:, 256 + o:256 + o + 128])
                n0 = gs.tile([128, 128], HF, tag=f"n00{o}")
                nc.scalar.activation(n0[:], t2[:], AF.Tanh)
                t0 = gs.tile([128, 128], HF, tag=f"t00{o}")
                nc.vector.scalar_tensor_tensor(
                    t0[:], rz[:, 128:256], 1.0, n0[:], ALU.subtract, ALU.mult)
                # h = z*h - (z-1)*n = z*h + (1-z)*n
                nc.vector.tensor_sub(h0[:, o:o + 128], zh[:], t0[:])
                nc.tensor.transpose(Ptr0[:, o:o + 128], h0[:, o:o + 128], ident16[:])
                if lo:
                    nc.scalar.copy(h0T[:, o:o + 128], Ptr0[:, o:o + 128])
                else:
                    nc.vector.tensor_copy(h0T[:, o:o + 128], Ptr0[:, o:o + 128])

            def l1_chain(Prz1, AB1):
                rz1 = gs.tile([128, 512], HF, tag="rz1")
                nc.scalar.activation(rz1[:], Prz1[:], AF.Sigmoid)
                zh1 = gs.tile([128, 256], HF, tag="zh1")
                nc.gpsimd.tensor_mul(zh1[:], rz1[:, 256:512], h1[:])
                t1b = gs.tile([128, 256], HF, tag="t1b")
                nc.vector.tensor_mul(t1b[:], rz1[:, 0:256], AB1[:, 0:256])
                t2b = gs.tile([128, 256], HF, tag="t2b")
                nc.vector.tensor_add(t2b[:], t1b[:], AB1[:, 256:512])
                n1 = gs.tile([128, 256], HF, tag="n1")
                nc.scalar.activation(n1[:], t2b[:], AF.Tanh)
                t0b = gs.tile([128, 256], HF, tag="t0b")
                nc.vector.scalar_tensor_tensor(
                    t0b[:], rz1[:, 256:512], 1.0, n1[:], ALU.subtract, ALU.mult)
                nc.gpsimd.tensor_sub(h1[:], zh1[:], t0b[:])

            def trs1():
                Ptr1 = gp.tile([128, 256], HF, tag="Ptr1")
                nc.tensor.transpose(Ptr1[:, 0:128], h1[:, 0:128], ident16[:])
                nc.tensor.transpose(Ptr1[:, 128:256], h1[:, 128:256], ident16[:])
                nc.scalar.copy(h1T[:], Ptr1[:])

            # L1 runs one step behind L0.
            pend = False
            for t in range(n_steps):
                Plo, Phi, AB0 = l0_mms(t)
                if pend:
                    Prz1, AB1 = l1_mms()
                Ptr0 = gp.tile([128, 256], HF, tag="Ptr0")
                l0_half(Plo, AB0, Ptr0, lo=True)
                l0_half(Phi, AB0, Ptr0, lo=False)
                if pend:
                    l1_chain(Prz1, AB1)
                    trs1()
                    fillers()
                pend = True
            # flush: L1 for the last step
            Prz1, AB1 = l1_mms()
            l1_chain(Prz1, AB1)
            trs1()

            # ---- projection ----
            Pout = gp.tile([128, 512], FP, tag="Prz1")
            mm(Pout[:, 0:NS], h1T[:, 0:128], pW[:, 0:NS], start=True, stop=False)
            mm(Pout[:, 0:NS], h1T[:, 128:256], pW[:, NS:2 * NS], start=False, stop=False)
            mm(Pout[:, 0:NS], ones1[:], pb[:], start=False, stop=True)
            out_sb = gs.tile([128, NS], FP, tag="outsb")
            nc.scalar.copy(out_sb[:], Pout[:, 0:NS])
            nc.gpsimd.dma_start(out_d[:], out_sb[:])

    _split_multi_waits(nc)
    return nc


# column permutation for layer-0 weights: (r_lo z_lo | r_hi z_hi | n_lo n_hi)
def _perm0():
    idx = np.arange(768)
    return np.concatenate([idx[0:128], idx[256:384], idx[128:256],
                           idx[384:512], idx[512:768]])


def prep_inputs(inputs):
    """Host-side shared (per-core-identical) tensor prep."""
    f = np.float32
    hf = np.float16
    g = {}
    bWf = np.asarray(inputs['branch_Wf'], f)      # (512, 528)
    bWr = np.asarray(inputs['branch_Wr'], f)      # (3, 512, 512)
    w = np.zeros((NBP, HID), f)
    w[:NB] = bWf.T
    g['bW0'] = w.reshape(5, 128, HID).astype(hf)
    for i in range(3):
        g[f'bW{i + 1}'] = np.ascontiguousarray(bWr[i].T).reshape(4, 128, HID).astype(hf)
    g['bb0'] = np.asarray(inputs['branch_bf'], f).reshape(4, 128).T.copy()
    for i in range(3):
        g[f'bb{i + 1}'] = np.asarray(inputs['branch_br'][i], f).reshape(4, 128).T.copy()
    g['tW0'] = np.asarray(inputs['trunk_Wf'], f).T.astype(hf)          # (1, 512)
    tWr = np.asarray(inputs['trunk_Wr'], f)
    for i in range(3):
        g[f'tW{i + 1}'] = np.ascontiguousarray(tWr[i].T).reshape(4, 128, HID).astype(hf)
    g['tb0'] = np.asarray(inputs['trunk_bf'], f).reshape(4, 128).T.copy()
    for i in range(3):
        g[f'tb{i + 1}'] = np.asarray(inputs['trunk_br'][i], f).reshape(4, 128).T.copy()
    g['tT'] = np.arange(T, dtype=f).reshape(1, T).astype(hf)

    p0 = _perm0()
    whh0 = np.ascontiguousarray(np.asarray(inputs['gru_Whh0'], f).T)   # (256, 768)
    g['whh0'] = whh0[:, p0].reshape(2, 128, 768).astype(hf)
    g['whh1'] = np.ascontiguousarray(np.asarray(inputs['gru_Whh1'], f).T).reshape(2, 128, 768).astype(hf)
    g['wih1'] = np.ascontiguousarray(np.asarray(inputs['gru_Wih1'], f).T).reshape(2, 128, 768).astype(hf)
    don = float(np.asarray(inputs['don_bias'], f).reshape(-1)[0])
    w0 = np.asarray(inputs['gru_Wih0'], f)[:, 0]  # (768,)
    bih0 = np.asarray(inputs['gru_bih0'], f)
    bhh0 = np.asarray(inputs['gru_bhh0'], f)
    aug0 = np.zeros((2, 768), f)
    aug0[0, :512] = w0[:512]
    aug0[1, :512] = bih0[:512] + bhh0[:512] + don * w0[:512]
    aug0[1, 512:768] = bhh0[512:768]
    g['aug0'] = aug0[:, p0].astype(hf)
    aug0n = np.zeros((2, 256), f)
    aug0n[0] = w0[512:768]
    aug0n[1] = bih0[512:768] + don * w0[512:768]
    g['aug0n'] = aug0n.astype(hf)
    bih1 = np.asarray(inputs['gru_bih1'], f)
    bhh1 = np.asarray(inputs['gru_bhh1'], f)
    aug1 = np.zeros((1, 768), f)
    aug1[0, :512] = bih1[:512] + bhh1[:512]
    aug1[0, 512:768] = bhh1[512:768]
    g['aug1'] = aug1.astype(hf)
    g['aug1n'] = bih1[512:768].reshape(1, 256).astype(hf)
    g['pW'] = np.ascontiguousarray(np.asarray(inputs['proj_W'], f).T).reshape(2, 128, NS).astype(hf)
    g['pb'] = np.asarray(inputs['proj_b'], f).reshape(1, NS).astype(hf)
    g['ident16'] = np.eye(128, dtype=hf)
    g['zer'] = np.zeros((128, GH), hf)
    g['ones16k'] = np.ones((1, T * BC), hf)
    return g


def run(inputs, **spmd_kwargs):
    from concourse.bass_utils import run_bass_kernel_spmd

    if 'nc' not in _CACHE:
        _CACHE['nc'] = build_nc(T)
    nc = _CACHE['nc']

    shared = prep_inputs(inputs)
    x = np.asarray(inputs['x'], np.float32)
    in_maps = []
    for c in range(NCORES):
        xs = x[c * BC:(c + 1) * BC]          # (128, 528)
        xt = np.zeros((NBP, BC), np.float32)
        xt[:NB] = xs.T
        m = dict(shared)
        m['xT'] = xt.reshape(5, 128, BC).astype(np.float16)
        in_maps.append(m)

    res = run_bass_kernel_spmd(nc, in_maps, list(range(NCORES)), **spmd_kwargs)
    out = np.concatenate([res.results[c]["out"] for c in range(NCORES)], axis=0)
    return out.astype(np.float32), res


def kernel(**inputs):
    out, _ = run(inputs)
    return out


if __name__ == "__main__":
    rng = np.random.RandomState(0)
    print("building nc...")
    nc = build_nc(2)
    print("built OK")


# revision 17
# speedup vs baseline: 1.1806x; 1.1806x over previous
"""DeepONet+GRU Trainium2 kernel (8-core data parallel), v3.

Full inputs in, full outputs out. Batch 1024 is sharded 128/core across 8
NeuronCores; all parameters are replicated. Per core:
  branch MLP (528->512 x4, fp16, transposed layout) -> branchT [512f, 128b]
  trunk MLP (1->512 x4, fp16) -> trunkT [512f, 128t]
  seq = branchT.T @ trunkT (+don_bias folded into GRU aug weights)
  2-layer GRU scan over T=128 steps, hidden 256, fully on-chip
  proj -> [128, 16]

v3 GRU structure:
  - all gate biases + input-side terms folded into PSUM via K=1/K=2
    matmuls (st = [seq_t; ones] rank-2 trick extended to the n-gate)
  - fp16 state + gates throughout
  - layer-0 gate chain split into GH-halves (wavefront): the lo-half of
    h0T is copied out while the hi-half chain still runs, so the next
    step's first recurrent matmuls start earlier. whh0/aug0 columns are
    host-permuted to (r_lo z_lo | r_hi z_hi | n_lo n_hi).
  - (z-1)*n fused via scalar_tensor_tensor; h = z*h - (z-1)*n
  - engine balance: ACT does sigmoids/tanhs + h1T copy, DVE does the
    mul/add chain + h0T half copies, Pool does z*h products and (z-1)*n
    for layer 0
  - low-priority junk filler matmuls let the scheduler keep the PE
    p-state ramped (PE runs 0.83ns/col after any idle gap, 0.417ns/col
    when continuously busy >3us)
"""
import sys
sys.path.insert(0, '/opt/trn_rl_repo')

import numpy as np

B = 1024
BC = 128          # batch per core
NB = 528
NBP = 640         # padded branch input (5 k-tiles)
HID = 512
GH = 256
T = 128
NS = 16
NCORES = 8

N_FILL = 6        # junk 256-col matmuls per step (PE p-state keep-alive)

_CACHE = {}


def _patched_tile_context(nc):
    """TileContext whose tail drain splits sem waits (walrus CoreV3 rejects
    >1 sync wait on a Drain)."""
    import concourse.tile as tile
    from concourse.vector_clock import ScopedClock

    class PatchedTileContext(tile.TileContext):
        def _drain_and_barrier(self, tick_clock, wait_clock):
            nc = self.nc
            drain_inst = nc.sync.drain()
            wait_clock.add_sem_waits(
                drain_inst.ins, ScopedClock({None: tick_clock.global_clock})
            )
            si = drain_inst.ins.sync_info
            waits = list(si.on_wait or []) if si is not None else []
            if len(waits) > 1:
                si.on_wait = waits[:1]
                for i in range(1, len(waits)):
                    extra = nc.sync.drain()
                    esi = extra.ins.sync_info
                    if esi is None:
                        from concourse import mybir
                        extra.ins.sync_info = mybir.SyncInfo(
                            on_wait=waits[i:i + 1], on_update=[]
                        )
                    else:
                        esi.on_wait = waits[i:i + 1]
            nc.all_engine_barrier()
            assert self.sems is not None
            popped = nc._tile_sem_poison_stack.pop()
            assert popped is self._sem_poison
            nc.clear_and_free_semaphores(list(self.sems.allocated().values()))
            nc.all_engine_barrier()

    return PatchedTileContext(nc)


def _split_multi_waits(nc):
    """This container's walrus rejects >1 sync wait per instruction
    ("Too many sync wait commands"). Hoist extra waits onto engine-matched
    NoOps spliced immediately before the offending instruction."""
    from concourse import mybir
    n_extra = 0
    for fn in nc.m.functions:
        for bb in fn.blocks:
            new = []
            for inst in bb.instructions:
                si = inst.sync_info
                waits = list(si.on_wait) if (si is not None and si.on_wait) else []
                if len(waits) > 1:
                    for w in waits[:-1]:
                        nop = mybir.InstNoOp(
                            name=f"wsplit-{n_extra}-{inst.name}",
                            engine=inst.engine,
                            bass_nofuse=True,
                            sync_info=mybir.SyncInfo(on_wait=[w], on_update=[]),
                        )
                        new.append(nop)
                        n_extra += 1
                    si.on_wait = [waits[-1]]
                new.append(inst)
            if n_extra:
                bb.instructions[:] = new
    return n_extra


def build_nc(n_steps=T):
    import concourse.bass as bass
    from concourse import mybir
    from contextlib import ExitStack

    FP = mybir.dt.float32
    HF = mybir.dt.float16
    AF = mybir.ActivationFunctionType
    ALU = mybir.AluOpType
    nc = bass.Bass()

    def mm(out, lhsT, rhs, start, stop):
        nc.tensor.matmul(out, lhsT, rhs, start=start, stop=stop)

    # ---- DRAM parameters (host-prepped layouts) ----
    dp = lambda name, shape, dt=FP: nc.declare_dram_parameter(name, list(shape), dt, isOutput=False)
    xT_d = dp("xT", (5, 128, BC), HF)
    bW_d = [dp("bW0", (5, 128, HID), HF)] + [dp(f"bW{i}", (4, 128, HID), HF) for i in (1, 2, 3)]
    bb_d = [dp(f"bb{i}", (128, 4)) for i in range(4)]
    tW0_d = dp("tW0", (1, HID), HF)
    tW_d = [None] + [dp(f"tW{i}", (4, 128, HID), HF) for i in (1, 2, 3)]
    tb_d = [dp(f"tb{i}", (128, 4)) for i in range(4)]
    tT_d = dp("tT", (1, T), HF)
    whh0_d = dp("whh0", (2, 128, 3 * GH), HF)
    whh1_d = dp("whh1", (2, 128, 3 * GH), HF)
    wih1_d = dp("wih1", (2, 128, 3 * GH), HF)
    aug0_d = dp("aug0", (2, 768), HF)
    aug0n_d = dp("aug0n", (2, 256), HF)
    aug1_d = dp("aug1", (1, 768), HF)
    aug1n_d = dp("aug1n", (1, 256), HF)
    pW_d = dp("pW", (2, 128, NS), HF)
    pb_d = dp("pb", (1, NS), HF)
    ident16_d = dp("ident16", (128, 128), HF)
    zer_d = dp("zer", (128, GH), HF)
    ones16k_d = dp("ones16k", (1, T * BC), HF)
    out_d = nc.declare_dram_parameter("out", [BC, NS], FP, isOutput=True)

    with ExitStack() as ctx:
        tc = ctx.enter_context(_patched_tile_context(nc))
        const = ctx.enter_context(tc.tile_pool(name="const", bufs=1))

        # ---- persistent SBUF ----
        ident16 = const.tile([128, 128], HF)
        nc.gpsimd.dma_start(ident16[:], ident16_d[:])
        whh0 = const.tile([128, 2 * 768], HF)
        whh1 = const.tile([128, 2 * 768], HF)
        wih1 = const.tile([128, 2 * 768], HF)
        for k in range(2):
            nc.gpsimd.dma_start(whh0[:, k * 768:(k + 1) * 768], whh0_d[k])
            nc.gpsimd.dma_start(whh1[:, k * 768:(k + 1) * 768], whh1_d[k])
            nc.gpsimd.dma_start(wih1[:, k * 768:(k + 1) * 768], wih1_d[k])
        aug0 = const.tile([2, 768], HF)
        nc.gpsimd.dma_start(aug0[:], aug0_d[:])
        aug0n = const.tile([2, 256], HF)
        nc.gpsimd.dma_start(aug0n[:], aug0n_d[:])
        aug1 = const.tile([1, 768], HF)
        nc.gpsimd.dma_start(aug1[:], aug1_d[:])
        aug1n = const.tile([1, 256], HF)
        nc.gpsimd.dma_start(aug1n[:], aug1n_d[:])
        pW = const.tile([128, 2 * NS], HF)
        for k in range(2):
            nc.gpsimd.dma_start(pW[:, k * NS:(k + 1) * NS], pW_d[k])
        pb = const.tile([1, NS], HF)
        nc.gpsimd.dma_start(pb[:], pb_d[:])
        ones1 = const.tile([1, 128], HF)
        nc.gpsimd.dma_start(ones1[:], ones16k_d[:, 0:128])

        branchT = const.tile([128, HID], HF)  # [feat within tile, 4 mtiles * batch]
        trunkT = const.tile([128, HID], HF)
        seqT_sb = const.tile([T, BC], HF)
        seq16 = const.tile([BC, T], HF)    # [batch, t] fp16 (transpose input)

        # states (fp16): h in [batch, GH]; hT in [GH-halves, batch]
        h0 = const.tile([128, GH], HF)
        h1 = const.tile([128, GH], HF)
        h0T = const.tile([128, GH], HF)
        h1T = const.tile([128, GH], HF)
        nc.vector.memset(h0[:], 0.0)
        nc.vector.memset(h1[:], 0.0)
        nc.gpsimd.dma_start(h0T[:], zer_d[:])
        nc.gpsimd.dma_start(h1T[:], zer_d[:])

        # ================= MLP phase (fp16) =================
        with tc.tile_pool(name="mlpw", bufs=1) as mlpw, \
             tc.tile_pool(name="mlps", bufs=2) as mlps, \
             tc.tile_pool(name="mlpp", bufs=4, space=bass.MemorySpace.PSUM) as mlpp:

            def mlp(xtiles_sb, nk_first, W_sbs, b_sbs, final_relu, out_sb):
                cur = xtiles_sb
                nlayers = 4
                for l in range(nlayers):
                    nk = nk_first if l == 0 else 4
                    Wl = W_sbs[l]
                    dst = out_sb if l == nlayers - 1 else mlps.tile([128, HID], HF, tag="mlpact")
                    for m in range(4):
                        ps = mlpp.tile([128, 128], FP, tag="mlppsum")
                        for k in range(nk):
                            mm(
                                ps[:],
                                Wl[:, k * HID + m * 128: k * HID + (m + 1) * 128],
                                cur[:, k * 128:(k + 1) * 128],
                                start=(k == 0), stop=(k == nk - 1),
                            )
                        func = AF.Relu if (l < nlayers - 1 or final_relu) else AF.Identity
                        nc.scalar.activation(
                            dst[:, m * 128:(m + 1) * 128], ps[:], func,
                            bias=b_sbs[l][:, m:m + 1],
                        )
                    cur = dst
                return cur

            # branch weights -> SBUF
            bW_sb = []
            for l in range(4):
                nk = 5 if l == 0 else 4
                w = mlpw.tile([128, nk * HID], HF, tag=f"bw{l}")
                for k in range(nk):
                    nc.gpsimd.dma_start(w[:, k * HID:(k + 1) * HID], bW_d[l][k])
                bW_sb.append(w)
            bb_sb = []
            for l in range(4):
                t_ = mlpw.tile([128, 4], FP, tag=f"bb{l}")
                nc.gpsimd.dma_start(t_[:], bb_d[l][:])
                bb_sb.append(t_)
            xk = mlpw.tile([128, 5 * 128], HF, tag="xk")
            for k in range(5):
                nc.gpsimd.dma_start(xk[:, k * 128:(k + 1) * 128], xT_d[k])
            mlp(xk, 5, bW_sb, bb_sb, final_relu=False, out_sb=branchT)

            # trunk: first layer K=1
            tW0 = mlpw.tile([1, HID], HF, tag="tw0")
            nc.gpsimd.dma_start(tW0[:], tW0_d[:])
            tTs = mlpw.tile([1, T], HF, tag="tts")
            nc.gpsimd.dma_start(tTs[:], tT_d[:])
            tb_sb = []
            for l in range(4):
                t_ = mlpw.tile([128, 4], FP, tag=f"tb{l}")
                nc.gpsimd.dma_start(t_[:], tb_d[l][:])
                tb_sb.append(t_)
            tW_sb = [None]
            for l in (1, 2, 3):
                w = mlpw.tile([128, 4 * HID], HF, tag=f"tw{l}")
                for k in range(4):
                    nc.gpsimd.dma_start(w[:, k * HID:(k + 1) * HID], tW_d[l][k])
                tW_sb.append(w)

            tact = mlps.tile([128, HID], HF, tag="mlpact")
            for m in range(4):
                ps = mlpp.tile([128, 128], FP, tag="mlppsum")
                mm(ps[:], tW0[:, m * 128:(m + 1) * 128], tTs[:],
                                 start=True, stop=True)
                nc.scalar.activation(tact[:, m * 128:(m + 1) * 128], ps[:],
                                     AF.Relu, bias=tb_sb[0][:, m:m + 1])
            # layers 1..3 of trunk
            cur = tact
            for l in (1, 2, 3):
                dst = trunkT if l == 3 else mlps.tile([128, HID], HF, tag="mlpact")
                for m in range(4):
                    ps = mlpp.tile([128, 128], FP, tag="mlppsum")
                    for k in range(4):
                        mm(
                            ps[:],
                            tW_sb[l][:, k * HID + m * 128: k * HID + (m + 1) * 128],
                            cur[:, k * 128:(k + 1) * 128],
                            start=(k == 0), stop=(k == 3),
                        )
                    nc.scalar.activation(dst[:, m * 128:(m + 1) * 128], ps[:],
                                         AF.Relu, bias=tb_sb[l][:, m:m + 1])
                cur = dst

            # seq[b,t] = sum_f branchT[f,b] * trunkT[f,t]  -> [B, T] psum
            ps_seq = mlpp.tile([128, 128], FP, tag="mlppsum")
            for k in range(4):
                mm(ps_seq[:], branchT[:, k * 128:(k + 1) * 128],
                                 trunkT[:, k * 128:(k + 1) * 128],
                                 start=(k == 0), stop=(k == 3))
            nc.scalar.copy(seq16[:], ps_seq[:])
            ps_seqT = mlpp.tile([128, 128], HF, tag="mlppsumT")
            nc.tensor.transpose(ps_seqT[:], seq16[:], ident16[:])
            nc.scalar.copy(seqT_sb[:], ps_seqT[:])

        # ================= GRU phase =================
        saug = const.tile([2, T * BC], HF)
        # partition-collapse seqT (t-major rows) into row 0 of saug
        nc.gpsimd.dma_start(saug[0:1, :], seqT_sb[:])
        nc.gpsimd.dma_start(saug[1:2, :], ones16k_d[:])

        with tc.tile_pool(name="gp", bufs=1, space=bass.MemorySpace.PSUM) as gp, \
             tc.tile_pool(name="gs", bufs=2) as gs:

            def l0_mms(t):
                # whh0/aug0 columns host-permuted: (r_lo z_lo | r_hi z_hi | n_lo n_hi)
                st = saug[:, t * BC:(t + 1) * BC]
                Plo = gp.tile([128, 256], FP, tag="Plo")    # (r_lo z_lo)
                mm(Plo[:], st, aug0[:, 0:256], start=True, stop=False)
                mm(Plo[:], h0T[:, 0:128], whh0[:, 0:256], start=False, stop=False)
                mm(Plo[:], h0T[:, 128:256], whh0[:, 768:1024], start=False, stop=True)
                Phi = gp.tile([128, 256], FP, tag="Phi")    # (r_hi z_hi)
                mm(Phi[:], st, aug0[:, 256:512], start=True, stop=False)
                mm(Phi[:], h0T[:, 0:128], whh0[:, 256:512], start=False, stop=False)
                mm(Phi[:], h0T[:, 128:256], whh0[:, 1024:1280], start=False, stop=True)
                AB0 = gp.tile([128, 512], FP, tag="AB0")    # (Bn_lo Bn_hi | A_lo A_hi)
                mm(AB0[:, 0:256], st, aug0[:, 512:768], start=True, stop=False)
                mm(AB0[:, 0:256], h0T[:, 0:128], whh0[:, 512:768], start=False, stop=False)
                mm(AB0[:, 0:256], h0T[:, 128:256], whh0[:, 1280:1536], start=False, stop=True)
                mm(AB0[:, 256:512], st, aug0n[:], start=True, stop=True)
                return Plo, Phi, AB0

            def l1_mms():
                # L1 for step t-1: h0T/h1T hold step t-1 / t-2 states here
                Prz1 = gp.tile([128, 512], FP, tag="Prz1")
                mm(Prz1[:], ones1[:], aug1[:, 0:512], start=True, stop=False)
                mm(Prz1[:], h1T[:, 0:128], whh1[:, 0:512], start=False, stop=False)
                mm(Prz1[:], h1T[:, 128:256], whh1[:, 768:1280], start=False, stop=False)
                mm(Prz1[:], h0T[:, 0:128], wih1[:, 0:512], start=False, stop=False)
                mm(Prz1[:], h0T[:, 128:256], wih1[:, 768:1280], start=False, stop=True)
                AB1 = gp.tile([128, 512], FP, tag="AB1")  # (B1n | A1n)
                mm(AB1[:, 0:256], ones1[:], aug1[:, 512:768], start=True, stop=False)
                mm(AB1[:, 0:256], h1T[:, 0:128], whh1[:, 512:768], start=False, stop=False)
                mm(AB1[:, 0:256], h1T[:, 128:256], whh1[:, 1280:1536], start=False, stop=True)
                mm(AB1[:, 256:512], ones1[:], aug1n[:], start=True, stop=False)
                mm(AB1[:, 256:512], h0T[:, 0:128], wih1[:, 512:768], start=False, stop=False)
                mm(AB1[:, 256:512], h0T[:, 128:256], wih1[:, 1280:1536], start=False, stop=True)
                return Prz1, AB1

            def l0_half(P, AB0, Ptr0, lo):
                # one GH-half of the layer-0 gate chain + transpose + copy
                o = 0 if lo else 128
                rz = gs.tile([128, 256], HF, tag=f"rz0{o}")
                nc.scalar.activation(rz[:], P[:], AF.Sigmoid)   # (r | z)
                zh = gs.tile([128, 128], HF, tag=f"zh0{o}")
                nc.gpsimd.tensor_mul(zh[:], rz[:, 128:256], h0[:, o:o + 128])
                t1 = gs.tile([128, 128], HF, tag=f"t10{o}")
                nc.vector.tensor_mul(t1[:], rz[:, 0:128], AB0[:, o:o + 128])
                t2 = gs.tile([128, 128], HF, tag=f"t20{o}")
                nc.vector.tensor_add(t2[:], t1[:], AB0[:, 256 + o:256 + o + 128])
                n0 = gs.tile([128, 128], HF, tag=f"n00{o}")
                nc.scalar.activation(n0[:], t2[:], AF.Tanh)
                t0 = gs.tile([128, 128], HF, tag=f"t00{o}")
                nc.vector.scalar_tensor_tensor(
                    t0[:], rz[:, 128:256], 1.0, n0[:], ALU.subtract, ALU.mult)
                # h = z*h - (z-1)*n = z*h + (1-z)*n
                nc.vector.tensor_sub(h0[:, o:o + 128], zh[:], t0[:])
                nc.tensor.transpose(Ptr0[:, o:o + 128], h0[:, o:o + 128], ident16[:])
                if lo:
                    nc.scalar.copy(h0T[:, o:o + 128], Ptr0[:, o:o + 128])
                else:
                    nc.vector.tensor_copy(h0T[:, o:o + 128], Ptr0[:, o:o + 128])

            def l1_chain(Prz1, AB1):
                rz1 = gs.tile([128, 512], HF, tag="rz1")
                nc.scalar.activation(rz1[:], Prz1[:], AF.Sigmoid)
                zh1 = gs.tile([128, 256], HF, tag="zh1")
                nc.gpsimd.tensor_mul(zh1[:], rz1[:, 256:512], h1[:])
                t1b = gs.tile([128, 256], HF, tag="t1b")
                nc.vector.tensor_mul(t1b[:], rz1[:, 0:256], AB1[:, 0:256])
                t2b = gs.tile([128, 256], HF, tag="t2b")
                nc.vector.tensor_add(t2b[:], t1b[:], AB1[:, 256:512])
                n1 = gs.tile([128, 256], HF, tag="n1")
                nc.scalar.activation(n1[:], t2b[:], AF.Tanh)
                t0b = gs.tile([128, 256], HF, tag="t0b")
                nc.vector.scalar_tensor_tensor(
                    t0b[:], rz1[:, 256:512], 1.0, n1[:], ALU.subtract, ALU.mult)
                nc.gpsimd.tensor_sub(h1[:], zh1[:], t0b[:])

            def trs1():
                Ptr1 = gp.tile([128, 256], HF, tag="Ptr1")
                nc.tensor.transpose(Ptr1[:, 0:128], h1[:, 0:128], ident16[:])
                nc.tensor.transpose(Ptr1[:, 128:256], h1[:, 128:256], ident16[:])
                nc.scalar.copy(h1T[:], Ptr1[:])

            # L1 runs one step behind L0.
            pend = False
            for t in range(n_steps):
                Plo, Phi, AB0 = l0_mms(t)
                if pend:
                    Prz1, AB1 = l1_mms()
                Ptr0 = gp.tile([128, 256], HF, tag="Ptr0")
                l0_half(Plo, AB0, Ptr0, lo=True)
                l0_half(Phi, AB0, Ptr0, lo=False)
                if pend:
                    l1_chain(Prz1, AB1)
                    trs1()
                pend = True
            # flush: L1 for the last step
            Prz1, AB1 = l1_mms()
            l1_chain(Prz1, AB1)
            trs1()

            # ---- projection ----
            Pout = gp.tile([128, 512], FP, tag="Prz1")
            mm(Pout[:, 0:NS], h1T[:, 0:128], pW[:, 0:NS], start=True, stop=False)
            mm(Pout[:, 0:NS], h1T[:, 128:256], pW[:, NS:2 * NS], start=False, stop=False)
            mm(Pout[:, 0:NS], ones1[:], pb[:], start=False, stop=True)
            out_sb = gs.tile([128, NS], FP, tag="outsb")
            nc.scalar.copy(out_sb[:], Pout[:, 0:NS])
            nc.gpsimd.dma_start(out_d[:], out_sb[:])

    _split_multi_waits(nc)
    return nc


# column permutation for layer-0 weights: (r_lo z_lo | r_hi z_hi | n_lo n_hi)
def _perm0():
    idx = np.arange(768)
    return np.concatenate([idx[0:128], idx[256:384], idx[128:256],
                           idx[384:512], idx[512:768]])


def prep_inputs(inputs):
    """Host-side shared (per-core-identical) tensor prep."""
    f = np.float32
    hf = np.float16
    g = {}
    bWf = np.asarray(inputs['branch_Wf'], f)      # (512, 528)
    bWr = np.asarray(inputs['branch_Wr'], f)      # (3, 512, 512)
    w = np.zeros((NBP, HID), f)
    w[:NB] = bWf.T
    g['bW0'] = w.reshape(5, 128, HID).astype(hf)
    for i in range(3):
        g[f'bW{i + 1}'] = np.ascontiguousarray(bWr[i].T).reshape(4, 128, HID).astype(hf)
    g['bb0'] = np.asarray(inputs['branch_bf'], f).reshape(4, 128).T.copy()
    for i in range(3):
        g[f'bb{i + 1}'] = np.asarray(inputs['branch_br'][i], f).reshape(4, 128).T.copy()
    g['tW0'] = np.asarray(inputs['trunk_Wf'], f).T.astype(hf)          # (1, 512)
    tWr = np.asarray(inputs['trunk_Wr'], f)
    for i in range(3):
        g[f'tW{i + 1}'] = np.ascontiguousarray(tWr[i].T).reshape(4, 128, HID).astype(hf)
    g['tb0'] = np.asarray(inputs['trunk_bf'], f).reshape(4, 128).T.copy()
    for i in range(3):
        g[f'tb{i + 1}'] = np.asarray(inputs['trunk_br'][i], f).reshape(4, 128).T.copy()
    g['tT'] = np.arange(T, dtype=f).reshape(1, T).astype(hf)

    p0 = _perm0()
    whh0 = np.ascontiguousarray(np.asarray(inputs['gru_Whh0'], f).T)   # (256, 768)
    g['whh0'] = whh0[:, p0].reshape(2, 128, 768).astype(hf)
    g['whh1'] = np.ascontiguousarray(np.asarray(inputs['gru_Whh1'], f).T).reshape(2, 128, 768).astype(hf)
    g['wih1'] = np.ascontiguousarray(np.asarray(inputs['gru_Wih1'], f).T).reshape(2, 128, 768).astype(hf)
    don = float(np.asarray(inputs['don_bias'], f).reshape(-1)[0])
    w0 = np.asarray(inputs['gru_Wih0'], f)[:, 0]  # (768,)
    bih0 = np.asarray(inputs['gru_bih0'], f)
    bhh0 = np.asarray(inputs['gru_bhh0'], f)
    aug0 = np.zeros((2, 768), f)
    aug0[0, :512] = w0[:512]
    aug0[1, :512] = bih0[:512] + bhh0[:512] + don * w0[:512]
    aug0[1, 512:768] = bhh0[512:768]
    g['aug0'] = aug0[:, p0].astype(hf)
    aug0n = np.zeros((2, 256), f)
    aug0n[0] = w0[512:768]
    aug0n[1] = bih0[512:768] + don * w0[512:768]
    g['aug0n'] = aug0n.astype(hf)
    bih1 = np.asarray(inputs['gru_bih1'], f)
    bhh1 = np.asarray(inputs['gru_bhh1'], f)
    aug1 = np.zeros((1, 768), f)
    aug1[0, :512] = bih1[:512] + bhh1[:512]
    aug1[0, 512:768] = bhh1[512:768]
    g['aug1'] = aug1.astype(hf)
    g['aug1n'] = bih1[512:768].reshape(1, 256).astype(hf)
    g['pW'] = np.ascontiguousarray(np.asarray(inputs['proj_W'], f).T).reshape(2, 128, NS).astype(hf)
    g['pb'] = np.asarray(inputs['proj_b'], f).reshape(1, NS).astype(hf)
    g['ident16'] = np.eye(128, dtype=hf)
    g['zer'] = np.zeros((128, GH), hf)
    g['ones16k'] = np.ones((1, T * BC), hf)
    return g


def run(inputs, **spmd_kwargs):
    from concourse.bass_utils import run_bass_kernel_spmd

    if 'nc' not in _CACHE:
        _CACHE['nc'] = build_nc(T)
    nc = _CACHE['nc']

    shared = prep_inputs(inputs)
    x = np.asarray(inputs['x'], np.float32)
    in_maps = []
    for c in range(NCORES):
        xs = x[c * BC:(c + 1) * BC]          # (128, 528)
        xt = np.zeros((NBP, BC), np.float32)
        xt[:NB] = xs.T
        m = dict(shared)
        m['xT'] = xt.reshape(5, 128, BC).astype(np.float16)
        in_maps.append(m)

    res = run_bass_kernel_spmd(nc, in_maps, list(range(NCORES)), **spmd_kwargs)
    out = np.concatenate([res.results[c]["out"] for c in range(NCORES)], axis=0)
    return out.astype(np.float32), res


def kernel(**inputs):
    out, _ = run(inputs)
    return out


if __name__ == "__main__":
    rng = np.random.RandomState(0)
    print("building nc...")
    nc = build_nc(2)
    print("built OK")


# revision 18
# speedup vs baseline: 1.4380x; 1.2180x over previous
"""DeepONet+GRU Trainium2 kernel (8-core data parallel), v5.

Full inputs in, full outputs out. Batch 1024 is sharded 128/core across 8
NeuronCores; all parameters are replicated. Per core:
  branch MLP (528->512 x4, fp16, transposed layout) -> branchT [512f, 128b]
  trunk MLP (1->512 x4, fp16) -> trunkT [512f, 128t]
  seq = branchT.T @ trunkT (+don_bias folded into GRU aug weights)
  2-layer GRU scan over T=128 steps, hidden 256, fully on-chip
  proj -> [128, 16]

GRU structure (empirically tuned):
  - big matmuls (512-col streams) + wide full-GH gate ops: fewer
    instructions sustain a higher tensor-engine clock on real HW than
    many small ones
  - all gate biases + input-side terms folded into PSUM via K=1/K=2
    matmuls (st = [seq_t; ones] rank-2 trick extended to the n-gate)
  - fp16 state + gates; (z-1)*n fused via scalar_tensor_tensor
  - junk filler matmuls sized to bridge the per-step PE idle windows;
    keeping the PE dense holds its DVFS clock high, which makes the
    real matmuls ~1.6x faster (measured)
  - MLP weight DMAs issued before GRU weight DMAs so the MLP phase is
    not starved behind 2.3MB of recurrent weights
"""
import sys
sys.path.insert(0, '/opt/trn_rl_repo')

import numpy as np

B = 1024
BC = 128          # batch per core
NB = 528
NBP = 640         # padded branch input (5 k-tiles)
HID = 512
GH = 256
T = 128
NS = 16
NCORES = 8

FILL1 = 1024      # junk cols after L1 mms, before trs0
FILL2 = 512       # after trs0, before trs1
FILL3 = 512       # after trs1, bridging into next iteration

_CACHE = {}


def _patched_tile_context(nc):
    """TileContext whose tail drain splits sem waits (walrus CoreV3 rejects
    >1 sync wait on a Drain)."""
    import concourse.tile as tile
    from concourse.vector_clock import ScopedClock

    class PatchedTileContext(tile.TileContext):
        def _drain_and_barrier(self, tick_clock, wait_clock):
            nc = self.nc
            drain_inst = nc.sync.drain()
            wait_clock.add_sem_waits(
                drain_inst.ins, ScopedClock({None: tick_clock.global_clock})
            )
            si = drain_inst.ins.sync_info
            waits = list(si.on_wait or []) if si is not None else []
            if len(waits) > 1:
                si.on_wait = waits[:1]
                for i in range(1, len(waits)):
                    extra = nc.sync.drain()
                    esi = extra.ins.sync_info
                    if esi is None:
                        from concourse import mybir
                        extra.ins.sync_info = mybir.SyncInfo(
                            on_wait=waits[i:i + 1], on_update=[]
                        )
                    else:
                        esi.on_wait = waits[i:i + 1]
            nc.all_engine_barrier()
            assert self.sems is not None
            popped = nc._tile_sem_poison_stack.pop()
            assert popped is self._sem_poison
            nc.clear_and_free_semaphores(list(self.sems.allocated().values()))
            nc.all_engine_barrier()

    return PatchedTileContext(nc)


def _split_multi_waits(nc):
    """This container's walrus rejects >1 sync wait per instruction
    ("Too many sync wait commands"). Hoist extra waits onto engine-matched
    NoOps spliced immediately before the offending instruction."""
    from concourse import mybir
    n_extra = 0
    for fn in nc.m.functions:
        for bb in fn.blocks:
            new = []
            for inst in bb.instructions:
                si = inst.sync_info
                waits = list(si.on_wait) if (si is not None and si.on_wait) else []
                if len(waits) > 1:
                    for w in waits[:-1]:
                        nop = mybir.InstNoOp(
                            name=f"wsplit-{n_extra}-{inst.name}",
                            engine=inst.engine,
                            bass_nofuse=True,
                            sync_info=mybir.SyncInfo(on_wait=[w], on_update=[]),
                        )
                        new.append(nop)
                        n_extra += 1
                    si.on_wait = [waits[-1]]
                new.append(inst)
            if n_extra:
                bb.instructions[:] = new
    return n_extra


def build_nc(n_steps=T):
    import concourse.bass as bass
    from concourse import mybir
    from contextlib import ExitStack

    FP = mybir.dt.float32
    HF = mybir.dt.float16
    AF = mybir.ActivationFunctionType
    ALU = mybir.AluOpType
    nc = bass.Bass()

    def mm(out, lhsT, rhs, start, stop):
        nc.tensor.matmul(out, lhsT, rhs, start=start, stop=stop)

    # ---- DRAM parameters (host-prepped layouts) ----
    dp = lambda name, shape, dt=FP: nc.declare_dram_parameter(name, list(shape), dt, isOutput=False)
    xT_d = dp("xT", (5, 128, BC), HF)
    bW_d = [dp("bW0", (5, 128, HID), HF)] + [dp(f"bW{i}", (4, 128, HID), HF) for i in (1, 2, 3)]
    bb_d = [dp(f"bb{i}", (128, 4)) for i in range(4)]
    tW0_d = dp("tW0", (1, HID), HF)
    tW_d = [None] + [dp(f"tW{i}", (4, 128, HID), HF) for i in (1, 2, 3)]
    tb_d = [dp(f"tb{i}", (128, 4)) for i in range(4)]
    tT_d = dp("tT", (1, T), HF)
    whh0_d = dp("whh0", (2, 128, 3 * GH), HF)
    whh1_d = dp("whh1", (2, 128, 3 * GH), HF)
    wih1_d = dp("wih1", (2, 128, 3 * GH), HF)
    aug0_d = dp("aug0", (2, 768), HF)
    aug0n_d = dp("aug0n", (2, 256), HF)
    aug1_d = dp("aug1", (1, 768), HF)
    aug1n_d = dp("aug1n", (1, 256), HF)
    pW_d = dp("pW", (2, 128, NS), HF)
    pb_d = dp("pb", (1, NS), HF)
    ident16_d = dp("ident16", (128, 128), HF)
    zer_d = dp("zer", (128, GH), HF)
    ones16k_d = dp("ones16k", (1, T * BC), HF)
    out_d = nc.declare_dram_parameter("out", [BC, NS], FP, isOutput=True)

    with ExitStack() as ctx:
        tc = ctx.enter_context(_patched_tile_context(nc))
        const = ctx.enter_context(tc.tile_pool(name="const", bufs=1))

        # ---- persistent SBUF (tiles allocated now, GRU weight DMAs issued
        # after the MLP weight DMAs so the MLP phase starts sooner) ----
        ident16 = const.tile([128, 128], HF)
        nc.gpsimd.dma_start(ident16[:], ident16_d[:])
        whh0 = const.tile([128, 2 * 768], HF)
        whh1 = const.tile([128, 2 * 768], HF)
        wih1 = const.tile([128, 2 * 768], HF)
        aug0 = const.tile([2, 768], HF)
        aug0n = const.tile([2, 256], HF)
        aug1 = const.tile([1, 768], HF)
        aug1n = const.tile([1, 256], HF)
        pW = const.tile([128, 2 * NS], HF)
        pb = const.tile([1, NS], HF)
        ones1 = const.tile([1, 128], HF)
        nc.gpsimd.dma_start(ones1[:], ones16k_d[:, 0:128])

        branchT = const.tile([128, HID], HF)  # [feat within tile, 4 mtiles * batch]
        trunkT = const.tile([128, HID], HF)
        seqT_sb = const.tile([T, BC], HF)
        seq16 = const.tile([BC, T], HF)    # [batch, t] fp16 (transpose input)

        # states (fp16): h in [batch, GH]; hT in [GH-halves, batch]
        h0 = const.tile([128, GH], HF)
        h1 = const.tile([128, GH], HF)
        h0T = const.tile([128, GH], HF)
        h1T = const.tile([128, GH], HF)
        nc.vector.memset(h0[:], 0.0)
        nc.vector.memset(h1[:], 0.0)
        nc.gpsimd.dma_start(h0T[:], zer_d[:])
        nc.gpsimd.dma_start(h1T[:], zer_d[:])

        def load_gru_weights():
            for k in range(2):
                nc.gpsimd.dma_start(whh0[:, k * 768:(k + 1) * 768], whh0_d[k])
                nc.gpsimd.dma_start(whh1[:, k * 768:(k + 1) * 768], whh1_d[k])
                nc.gpsimd.dma_start(wih1[:, k * 768:(k + 1) * 768], wih1_d[k])
            nc.gpsimd.dma_start(aug0[:], aug0_d[:])
            nc.gpsimd.dma_start(aug0n[:], aug0n_d[:])
            nc.gpsimd.dma_start(aug1[:], aug1_d[:])
            nc.gpsimd.dma_start(aug1n[:], aug1n_d[:])
            for k in range(2):
                nc.gpsimd.dma_start(pW[:, k * NS:(k + 1) * NS], pW_d[k])
            nc.gpsimd.dma_start(pb[:], pb_d[:])

        # ================= MLP phase (fp16) =================
        with tc.tile_pool(name="mlpw", bufs=1) as mlpw, \
             tc.tile_pool(name="mlps", bufs=2) as mlps, \
             tc.tile_pool(name="mlpp", bufs=4, space=bass.MemorySpace.PSUM) as mlpp:

            def mlp(xtiles_sb, nk_first, W_sbs, b_sbs, final_relu, out_sb):
                cur = xtiles_sb
                nlayers = 4
                for l in range(nlayers):
                    nk = nk_first if l == 0 else 4
                    Wl = W_sbs[l]
                    dst = out_sb if l == nlayers - 1 else mlps.tile([128, HID], HF, tag="mlpact")
                    for m in range(4):
                        ps = mlpp.tile([128, 128], FP, tag="mlppsum")
                        for k in range(nk):
                            mm(
                                ps[:],
                                Wl[:, k * HID + m * 128: k * HID + (m + 1) * 128],
                                cur[:, k * 128:(k + 1) * 128],
                                start=(k == 0), stop=(k == nk - 1),
                            )
                        func = AF.Relu if (l < nlayers - 1 or final_relu) else AF.Identity
                        nc.scalar.activation(
                            dst[:, m * 128:(m + 1) * 128], ps[:], func,
                            bias=b_sbs[l][:, m:m + 1],
                        )
                    cur = dst
                return cur

            # branch weights -> SBUF
            bW_sb = []
            for l in range(4):
                nk = 5 if l == 0 else 4
                w = mlpw.tile([128, nk * HID], HF, tag=f"bw{l}")
                for k in range(nk):
                    nc.gpsimd.dma_start(w[:, k * HID:(k + 1) * HID], bW_d[l][k])
                bW_sb.append(w)
            bb_sb = []
            for l in range(4):
                t_ = mlpw.tile([128, 4], FP, tag=f"bb{l}")
                nc.gpsimd.dma_start(t_[:], bb_d[l][:])
                bb_sb.append(t_)
            xk = mlpw.tile([128, 5 * 128], HF, tag="xk")
            for k in range(5):
                nc.gpsimd.dma_start(xk[:, k * 128:(k + 1) * 128], xT_d[k])

            # trunk weights -> SBUF
            tW0 = mlpw.tile([1, HID], HF, tag="tw0")
            nc.gpsimd.dma_start(tW0[:], tW0_d[:])
            tTs = mlpw.tile([1, T], HF, tag="tts")
            nc.gpsimd.dma_start(tTs[:], tT_d[:])
            tb_sb = []
            for l in range(4):
                t_ = mlpw.tile([128, 4], FP, tag=f"tb{l}")
                nc.gpsimd.dma_start(t_[:], tb_d[l][:])
                tb_sb.append(t_)
            tW_sb = [None]
            for l in (1, 2, 3):
                w = mlpw.tile([128, 4 * HID], HF, tag=f"tw{l}")
                for k in range(4):
                    nc.gpsimd.dma_start(w[:, k * HID:(k + 1) * HID], tW_d[l][k])
                tW_sb.append(w)

            # GRU weights queued behind all MLP weights
            load_gru_weights()

            mlp(xk, 5, bW_sb, bb_sb, final_relu=False, out_sb=branchT)

            tact = mlps.tile([128, HID], HF, tag="mlpact")
            for m in range(4):
                ps = mlpp.tile([128, 128], FP, tag="mlppsum")
                mm(ps[:], tW0[:, m * 128:(m + 1) * 128], tTs[:],
                                 start=True, stop=True)
                nc.scalar.activation(tact[:, m * 128:(m + 1) * 128], ps[:],
                                     AF.Relu, bias=tb_sb[0][:, m:m + 1])
            # layers 1..3 of trunk
            cur = tact
            for l in (1, 2, 3):
                dst = trunkT if l == 3 else mlps.tile([128, HID], HF, tag="mlpact")
                for m in range(4):
                    ps = mlpp.tile([128, 128], FP, tag="mlppsum")
                    for k in range(4):
                        mm(
                            ps[:],
                            tW_sb[l][:, k * HID + m * 128: k * HID + (m + 1) * 128],
                            cur[:, k * 128:(k + 1) * 128],
                            start=(k == 0), stop=(k == 3),
                        )
                    nc.scalar.activation(dst[:, m * 128:(m + 1) * 128], ps[:],
                                         AF.Relu, bias=tb_sb[l][:, m:m + 1])
                cur = dst

            # seq[b,t] = sum_f branchT[f,b] * trunkT[f,t]  -> [B, T] psum
            ps_seq = mlpp.tile([128, 128], FP, tag="mlppsum")
            for k in range(4):
                mm(ps_seq[:], branchT[:, k * 128:(k + 1) * 128],
                                 trunkT[:, k * 128:(k + 1) * 128],
                                 start=(k == 0), stop=(k == 3))
            nc.scalar.copy(seq16[:], ps_seq[:])
            ps_seqT = mlpp.tile([128, 128], HF, tag="mlppsumT")
            nc.tensor.transpose(ps_seqT[:], seq16[:], ident16[:])
            nc.scalar.copy(seqT_sb[:], ps_seqT[:])

        # ================= GRU phase =================
        saug = const.tile([2, T * BC], HF)
        # partition-collapse seqT (t-major rows) into row 0 of saug
        nc.gpsimd.dma_start(saug[0:1, :], seqT_sb[:])
        nc.gpsimd.dma_start(saug[1:2, :], ones16k_d[:])

        with tc.tile_pool(name="gp", bufs=1, space=bass.MemorySpace.PSUM) as gp, \
             tc.tile_pool(name="gpt", bufs=1, space=bass.MemorySpace.PSUM) as gpt, \
             tc.tile_pool(name="gs", bufs=2) as gs:

            Pfill = gp.tile([128, 512], FP, tag="Pfill")

            def filler(cols):
                done = 0
                while done < cols:
                    n = min(512, cols - done)
                    mm(Pfill[:, 0:n], ident16[:], whh0[:, 0:n],
                       start=True, stop=True)
                    done += n

            def l0_mms(t):
                st = saug[:, t * BC:(t + 1) * BC]
                Prz0 = gp.tile([128, 512], FP, tag="Prz0")
                mm(Prz0[:], st, aug0[:, 0:512], start=True, stop=False)
                mm(Prz0[:], h0T[:, 0:128], whh0[:, 0:512], start=False, stop=False)
                mm(Prz0[:], h0T[:, 128:256], whh0[:, 768:1280], start=False, stop=True)
                AB0 = gp.tile([128, 512], FP, tag="AB0")  # (B0n | A0n)
                mm(AB0[:, 0:256], st, aug0[:, 512:768], start=True, stop=False)
                mm(AB0[:, 0:256], h0T[:, 0:128], whh0[:, 512:768], start=False, stop=False)
                mm(AB0[:, 0:256], h0T[:, 128:256], whh0[:, 1280:1536], start=False, stop=True)
                mm(AB0[:, 256:512], st, aug0n[:], start=True, stop=True)
                return Prz0, AB0

            def l1_mms():
                # L1 for step t-1: h0T/h1T hold step t-1 / t-2 states here
                Prz1 = gp.tile([128, 512], FP, tag="Prz1")
                mm(Prz1[:], ones1[:], aug1[:, 0:512], start=True, stop=False)
                mm(Prz1[:], h1T[:, 0:128], whh1[:, 0:512], start=False, stop=False)
                mm(Prz1[:], h1T[:, 128:256], whh1[:, 768:1280], start=False, stop=False)
                mm(Prz1[:], h0T[:, 0:128], wih1[:, 0:512], start=False, stop=False)
                mm(Prz1[:], h0T[:, 128:256], wih1[:, 768:1280], start=False, stop=True)
                AB1 = gp.tile([128, 512], FP, tag="AB1")  # (B1n | A1n)
                mm(AB1[:, 0:256], ones1[:], aug1[:, 512:768], start=True, stop=False)
                mm(AB1[:, 0:256], h1T[:, 0:128], whh1[:, 512:768], start=False, stop=False)
                mm(AB1[:, 0:256], h1T[:, 128:256], whh1[:, 1280:1536], start=False, stop=True)
                mm(AB1[:, 256:512], ones1[:], aug1n[:], start=True, stop=False)
                mm(AB1[:, 256:512], h0T[:, 0:128], wih1[:, 512:768], start=False, stop=False)
                mm(AB1[:, 256:512], h0T[:, 128:256], wih1[:, 1280:1536], start=False, stop=True)
                return Prz1, AB1

            def l0_chain(Prz0, AB0):
                rz0 = gs.tile([128, 512], HF, tag="rz0")
                nc.scalar.activation(rz0[:], Prz0[:], AF.Sigmoid)
                zh0 = gs.tile([128, 256], HF, tag="zh0")
                nc.gpsimd.tensor_mul(zh0[:], rz0[:, 256:512], h0[:])
                t1 = gs.tile([128, 256], HF, tag="t1")
                nc.vector.tensor_mul(t1[:], rz0[:, 0:256], AB0[:, 0:256])
                t2 = gs.tile([128, 256], HF, tag="t2")
                nc.vector.tensor_add(t2[:], t1[:], AB0[:, 256:512])
                n0 = gs.tile([128, 256], HF, tag="n0")
                nc.scalar.activation(n0[:], t2[:], AF.Tanh)
                t0 = gs.tile([128, 256], HF, tag="t0")
                nc.vector.scalar_tensor_tensor(
                    t0[:], rz0[:, 256:512], 1.0, n0[:], ALU.subtract, ALU.mult)
                # h = z*h - (z-1)*n = z*h + (1-z)*n
                nc.vector.tensor_sub(h0[:], zh0[:], t0[:])

            def l1_chain(Prz1, AB1):
                rz1 = gs.tile([128, 512], HF, tag="rz1")
                nc.scalar.activation(rz1[:], Prz1[:], AF.Sigmoid)
                zh1 = gs.tile([128, 256], HF, tag="zh1")
                nc.gpsimd.tensor_mul(zh1[:], rz1[:, 256:512], h1[:])
                t1b = gs.tile([128, 256], HF, tag="t1b")
                nc.vector.tensor_mul(t1b[:], rz1[:, 0:256], AB1[:, 0:256])
                t2b = gs.tile([128, 256], HF, tag="t2b")
                nc.vector.tensor_add(t2b[:], t1b[:], AB1[:, 256:512])
                n1 = gs.tile([128, 256], HF, tag="n1")
                nc.scalar.activation(n1[:], t2b[:], AF.Tanh)
                t0b = gs.tile([128, 256], HF, tag="t0b")
                nc.vector.scalar_tensor_tensor(
                    t0b[:], rz1[:, 256:512], 1.0, n1[:], ALU.subtract, ALU.mult)
                nc.gpsimd.tensor_sub(h1[:], zh1[:], t0b[:])

            def trs0():
                Ptr0 = gpt.tile([128, 256], HF, tag="Ptr0")
                nc.tensor.transpose(Ptr0[:, 0:128], h0[:, 0:128], ident16[:])
                nc.tensor.transpose(Ptr0[:, 128:256], h0[:, 128:256], ident16[:])
                nc.scalar.copy(h0T[:], Ptr0[:])

            def trs1():
                Ptr1 = gpt.tile([128, 256], HF, tag="Ptr1")
                nc.tensor.transpose(Ptr1[:, 0:128], h1[:, 0:128], ident16[:])
                nc.tensor.transpose(Ptr1[:, 128:256], h1[:, 128:256], ident16[:])
                nc.vector.tensor_copy(h1T[:], Ptr1[:])

            # L1 runs one step behind L0.
            pend = False
            for t in range(n_steps):
                Prz0, AB0 = l0_mms(t)
                if pend:
                    Prz1, AB1 = l1_mms()
                l0_chain(Prz0, AB0)
                if pend:
                    filler(FILL1)
                trs0()
                if pend:
                    l1_chain(Prz1, AB1)
                    filler(FILL2)
                    trs1()
                    filler(FILL3)
                pend = True
            # flush: L1 for the last step
            Prz1, AB1 = l1_mms()
            l1_chain(Prz1, AB1)
            trs1()

            # ---- projection ----
            Pout = gp.tile([128, 512], FP, tag="Prz1")
            mm(Pout[:, 0:NS], h1T[:, 0:128], pW[:, 0:NS], start=True, stop=False)
            mm(Pout[:, 0:NS], h1T[:, 128:256], pW[:, NS:2 * NS], start=False, stop=False)
            mm(Pout[:, 0:NS], ones1[:], pb[:], start=False, stop=True)
            out_sb = gs.tile([128, NS], FP, tag="outsb")
            nc.scalar.copy(out_sb[:], Pout[:, 0:NS])
            nc.gpsimd.dma_start(out_d[:], out_sb[:])

    _split_multi_waits(nc)
    return nc


def prep_inputs(inputs):
    """Host-side shared (per-core-identical) tensor prep."""
    f = np.float32
    hf = np.float16
    g = {}
    bWf = np.asarray(inputs['branch_Wf'], f)      # (512, 528)
    bWr = np.asarray(inputs['branch_Wr'], f)      # (3, 512, 512)
    w = np.zeros((NBP, HID), f)
    w[:NB] = bWf.T
    g['bW0'] = w.reshape(5, 128, HID).astype(hf)
    for i in range(3):
        g[f'bW{i + 1}'] = np.ascontiguousarray(bWr[i].T).reshape(4, 128, HID).astype(hf)
    g['bb0'] = np.asarray(inputs['branch_bf'], f).reshape(4, 128).T.copy()
    for i in range(3):
        g[f'bb{i + 1}'] = np.asarray(inputs['branch_br'][i], f).reshape(4, 128).T.copy()
    g['tW0'] = np.asarray(inputs['trunk_Wf'], f).T.astype(hf)          # (1, 512)
    tWr = np.asarray(inputs['trunk_Wr'], f)
    for i in range(3):
        g[f'tW{i + 1}'] = np.ascontiguousarray(tWr[i].T).reshape(4, 128, HID).astype(hf)
    g['tb0'] = np.asarray(inputs['trunk_bf'], f).reshape(4, 128).T.copy()
    for i in range(3):
        g[f'tb{i + 1}'] = np.asarray(inputs['trunk_br'][i], f).reshape(4, 128).T.copy()
    g['tT'] = np.arange(T, dtype=f).reshape(1, T).astype(hf)

    g['whh0'] = np.ascontiguousarray(np.asarray(inputs['gru_Whh0'], f).T).reshape(2, 128, 768).astype(hf)
    g['whh1'] = np.ascontiguousarray(np.asarray(inputs['gru_Whh1'], f).T).reshape(2, 128, 768).astype(hf)
    g['wih1'] = np.ascontiguousarray(np.asarray(inputs['gru_Wih1'], f).T).reshape(2, 128, 768).astype(hf)
    don = float(np.asarray(inputs['don_bias'], f).reshape(-1)[0])
    w0 = np.asarray(inputs['gru_Wih0'], f)[:, 0]  # (768,)
    bih0 = np.asarray(inputs['gru_bih0'], f)
    bhh0 = np.asarray(inputs['gru_bhh0'], f)
    aug0 = np.zeros((2, 768), f)
    aug0[0, :512] = w0[:512]
    aug0[1, :512] = bih0[:512] + bhh0[:512] + don * w0[:512]
    aug0[1, 512:768] = bhh0[512:768]
    g['aug0'] = aug0.astype(hf)
    aug0n = np.zeros((2, 256), f)
    aug0n[0] = w0[512:768]
    aug0n[1] = bih0[512:768] + don * w0[512:768]
    g['aug0n'] = aug0n.astype(hf)
    bih1 = np.asarray(inputs['gru_bih1'], f)
    bhh1 = np.asarray(inputs['gru_bhh1'], f)
    aug1 = np.zeros((1, 768), f)
    aug1[0, :512] = bih1[:512] + bhh1[:512]
    aug1[0, 512:768] = bhh1[512:768]
    g['aug1'] = aug1.astype(hf)
    g['aug1n'] = bih1[512:768].reshape(1, 256).astype(hf)
    g['pW'] = np.ascontiguousarray(np.asarray(inputs['proj_W'], f).T).reshape(2, 128, NS).astype(hf)
    g['pb'] = np.asarray(inputs['proj_b'], f).reshape(1, NS).astype(hf)
    g['ident16'] = np.eye(128, dtype=hf)
    g['zer'] = np.zeros((128, GH), hf)
    g['ones16k'] = np.ones((1, T * BC), hf)
    return g


def run(inputs, **spmd_kwargs):
    from concourse.bass_utils import run_bass_kernel_spmd

    if 'nc' not in _CACHE:
        _CACHE['nc'] = build_nc(T)
    nc = _CACHE['nc']

    shared = prep_inputs(inputs)
    x = np.asarray(inputs['x'], np.float32)
    in_maps = []
    for c in range(NCORES):
        xs = x[c * BC:(c + 1) * BC]          # (128, 528)
        xt = np.zeros((NBP, BC), np.float32)
        xt[:NB] = xs.T
        m = dict(shared)
        m['xT'] = xt.reshape(5, 128, BC).astype(np.float16)
        in_maps.append(m)

    res = run_bass_kernel_spmd(nc, in_maps, list(range(NCORES)), **spmd_kwargs)
    out = np.concatenate([res.results[c]["out"] for c in range(NCORES)], axis=0)
    return out.astype(np.float32), res


def kernel(**inputs):
    out, _ = run(inputs)
    return out


if __name__ == "__main__":
    rng = np.random.RandomState(0)
    print("building nc...")
    nc = build_nc(2)
    print("built OK")


# revision 29
# speedup vs baseline: 1.4637x; 1.0178x over previous
"""DeepONet+GRU Trainium2 kernel (8-core data parallel), v5.

Full inputs in, full outputs out. Batch 1024 is sharded 128/core across 8
NeuronCores; all parameters are replicated. Per core:
  branch MLP (528->512 x4, fp16, transposed layout) -> branchT [512f, 128b]
  trunk MLP (1->512 x4, fp16) -> trunkT [512f, 128t]
  seq = branchT.T @ trunkT (+don_bias folded into GRU aug weights)
  2-layer GRU scan over T=128 steps, hidden 256, fully on-chip
  proj -> [128, 16]

GRU structure (empirically tuned):
  - big matmuls (512-col streams) + wide full-GH gate ops: fewer
    instructions sustain a higher tensor-engine clock on real HW than
    many small ones
  - all gate biases + input-side terms folded into PSUM via K=1/K=2
    matmuls (st = [seq_t; ones] rank-2 trick extended to the n-gate)
  - fp16 state + gates; (z-1)*n fused via scalar_tensor_tensor
  - junk filler matmuls sized to bridge the per-step PE idle windows;
    keeping the PE dense holds its DVFS clock high, which makes the
    real matmuls ~1.6x faster (measured)
  - MLP weight DMAs issued before GRU weight DMAs so the MLP phase is
    not starved behind 2.3MB of recurrent weights
"""
import sys
sys.path.insert(0, '/opt/trn_rl_repo')

import numpy as np

B = 1024
BC = 128          # batch per core
NB = 528
NBP = 640         # padded branch input (5 k-tiles)
HID = 512
GH = 256
T = 128
NS = 16
NCORES = 8

FILL1 = 1024      # junk cols after L1 mms, before trs0
FILL2 = 512       # after trs0, before trs1
FILL3 = 512       # after trs1, bridging into next iteration

_CACHE = {}


def _patched_tile_context(nc):
    """TileContext whose tail drain splits sem waits (walrus CoreV3 rejects
    >1 sync wait on a Drain)."""
    import concourse.tile as tile
    from concourse.vector_clock import ScopedClock

    class PatchedTileContext(tile.TileContext):
        def _drain_and_barrier(self, tick_clock, wait_clock):
            nc = self.nc
            drain_inst = nc.sync.drain()
            wait_clock.add_sem_waits(
                drain_inst.ins, ScopedClock({None: tick_clock.global_clock})
            )
            si = drain_inst.ins.sync_info
            waits = list(si.on_wait or []) if si is not None else []
            if len(waits) > 1:
                si.on_wait = waits[:1]
                for i in range(1, len(waits)):
                    extra = nc.sync.drain()
                    esi = extra.ins.sync_info
                    if esi is None:
                        from concourse import mybir
                        extra.ins.sync_info = mybir.SyncInfo(
                            on_wait=waits[i:i + 1], on_update=[]
                        )
                    else:
                        esi.on_wait = waits[i:i + 1]
            nc.all_engine_barrier()
            assert self.sems is not None
            popped = nc._tile_sem_poison_stack.pop()
            assert popped is self._sem_poison
            nc.clear_and_free_semaphores(list(self.sems.allocated().values()))
            nc.all_engine_barrier()

    return PatchedTileContext(nc)


def _split_multi_waits(nc):
    """This container's walrus rejects >1 sync wait per instruction
    ("Too many sync wait commands"). Hoist extra waits onto engine-matched
    NoOps spliced immediately before the offending instruction."""
    from concourse import mybir
    n_extra = 0
    for fn in nc.m.functions:
        for bb in fn.blocks:
            new = []
            for inst in bb.instructions:
                si = inst.sync_info
                waits = list(si.on_wait) if (si is not None and si.on_wait) else []
                if len(waits) > 1:
                    for w in waits[:-1]:
                        nop = mybir.InstNoOp(
                            name=f"wsplit-{n_extra}-{inst.name}",
                            engine=inst.engine,
                            bass_nofuse=True,
                            sync_info=mybir.SyncInfo(on_wait=[w], on_update=[]),
                        )
                        new.append(nop)
                        n_extra += 1
                    si.on_wait = [waits[-1]]
                new.append(inst)
            if n_extra:
                bb.instructions[:] = new
    return n_extra


def build_nc(n_steps=T):
    import concourse.bass as bass
    from concourse import mybir
    from contextlib import ExitStack

    FP = mybir.dt.float32
    HF = mybir.dt.float16
    AF = mybir.ActivationFunctionType
    ALU = mybir.AluOpType
    nc = bass.Bass()

    def mm(out, lhsT, rhs, start, stop):
        nc.tensor.matmul(out, lhsT, rhs, start=start, stop=stop)

    # ---- DRAM parameters (host-prepped layouts) ----
    dp = lambda name, shape, dt=FP: nc.declare_dram_parameter(name, list(shape), dt, isOutput=False)
    xT_d = dp("xT", (128, 5 * BC), HF)
    bW_d = [dp("bW0", (128, 5 * HID), HF)] + [dp(f"bW{i}", (128, 4 * HID), HF) for i in (1, 2, 3)]
    bb_d = [dp(f"bb{i}", (128, 4)) for i in range(4)]
    tW0_d = dp("tW0", (1, HID), HF)
    tW_d = [None] + [dp(f"tW{i}", (128, 4 * HID), HF) for i in (1, 2, 3)]
    tb_d = [dp(f"tb{i}", (128, 4)) for i in range(4)]
    tT_d = dp("tT", (1, T), HF)
    whh0_d = dp("whh0", (128, 2 * 768), HF)
    whh1_d = dp("whh1", (128, 2 * 768), HF)
    wih1_d = dp("wih1", (128, 2 * 768), HF)
    aug0_d = dp("aug0", (2, 768), HF)
    aug0n_d = dp("aug0n", (2, 256), HF)
    aug1_d = dp("aug1", (1, 768), HF)
    aug1n_d = dp("aug1n", (1, 256), HF)
    pW_d = dp("pW", (128, 2 * NS), HF)
    pb_d = dp("pb", (1, NS), HF)
    ident16_d = dp("ident16", (128, 128), HF)
    zer_d = dp("zer", (128, GH), HF)
    ones16k_d = dp("ones16k", (1, T * BC), HF)
    out_d = nc.declare_dram_parameter("out", [BC, NS], FP, isOutput=True)

    with ExitStack() as ctx:
        tc = ctx.enter_context(_patched_tile_context(nc))
        const = ctx.enter_context(tc.tile_pool(name="const", bufs=1))

        # ---- persistent SBUF (tiles allocated now, GRU weight DMAs issued
        # after the MLP weight DMAs so the MLP phase starts sooner) ----
        ident16 = const.tile([128, 128], HF)
        nc.gpsimd.dma_start(ident16[:], ident16_d[:])
        whh0 = const.tile([128, 2 * 768], HF)
        whh1 = const.tile([128, 2 * 768], HF)
        wih1 = const.tile([128, 2 * 768], HF)
        aug0 = const.tile([2, 768], HF)
        aug0n = const.tile([2, 256], HF)
        aug1 = const.tile([1, 768], HF)
        aug1n = const.tile([1, 256], HF)
        pW = const.tile([128, 2 * NS], HF)
        pb = const.tile([1, NS], HF)
        ones1 = const.tile([1, 128], HF)
        nc.gpsimd.dma_start(ones1[:], ones16k_d[:, 0:128])

        branchT = const.tile([128, HID], HF)  # [feat within tile, 4 mtiles * batch]
        trunkT = const.tile([128, HID], HF)
        seqT_sb = const.tile([T, BC], HF)
        seq16 = const.tile([BC, T], HF)    # [batch, t] fp16 (transpose input)

        # states (fp16): h in [batch, GH]; hT in [GH-halves, batch]
        h0 = const.tile([128, GH], HF)
        h1 = const.tile([128, GH], HF)
        h0T = const.tile([128, GH], HF)
        h1T = const.tile([128, GH], HF)
        nc.vector.memset(h0[:], 0.0)
        nc.vector.memset(h1[:], 0.0)
        nc.gpsimd.dma_start(h0T[:], zer_d[:])
        nc.gpsimd.dma_start(h1T[:], zer_d[:])

        def load_gru_weights():
            nc.gpsimd.dma_start(whh0[:], whh0_d[:])
            nc.sync.dma_start(whh1[:], whh1_d[:])
            nc.scalar.dma_start(wih1[:], wih1_d[:])
            nc.gpsimd.dma_start(aug0[:], aug0_d[:])
            nc.gpsimd.dma_start(aug0n[:], aug0n_d[:])
            nc.sync.dma_start(aug1[:], aug1_d[:])
            nc.sync.dma_start(aug1n[:], aug1n_d[:])
            nc.scalar.dma_start(pW[:], pW_d[:])
            nc.scalar.dma_start(pb[:], pb_d[:])

        # ================= MLP phase (fp16) =================
        with tc.tile_pool(name="mlpw", bufs=1) as mlpw, \
             tc.tile_pool(name="mlps", bufs=2) as mlps, \
             tc.tile_pool(name="mlpp", bufs=2, space=bass.MemorySpace.PSUM) as mlpp:

            def mlp_layer(cur, nk, Wl, bl, func, out_sb):
                # out[b, fo] = cur(hT).T @ Wl, streamed 512 wide; then
                # transpose back to [fo, b] and apply bias+act per m-chunk
                ps = mlpp.tile([128, HID], FP, tag="mlpps")
                for k in range(nk):
                    mm(ps[:], cur[:, k * 128:(k + 1) * 128],
                       Wl[:, k * HID:(k + 1) * HID],
                       start=(k == 0), stop=(k == nk - 1))
                sb = mlps.tile([128, HID], HF, tag="mlpsb")
                nc.vector.tensor_copy(sb[:], ps[:])
                pt = mlpp.tile([128, HID], HF, tag="mlppt")
                for m in range(4):
                    nc.tensor.transpose(pt[:, m * 128:(m + 1) * 128],
                                        sb[:, m * 128:(m + 1) * 128], ident16[:])
                for m in range(4):
                    nc.scalar.activation(
                        out_sb[:, m * 128:(m + 1) * 128],
                        pt[:, m * 128:(m + 1) * 128], func,
                        bias=bl[:, m:m + 1])
                return out_sb

            # branch weights -> SBUF (single-shot DMAs spread over queues)
            qs = [nc.gpsimd, nc.sync, nc.scalar]
            bW_sb = []
            for l in range(4):
                nk = 5 if l == 0 else 4
                w = mlpw.tile([128, nk * HID], HF, tag=f"bw{l}")
                qs[l % 3].dma_start(w[:], bW_d[l][:])
                bW_sb.append(w)
            bb_sb = []
            for l in range(4):
                t_ = mlpw.tile([128, 4], FP, tag=f"bb{l}")
                qs[l % 3].dma_start(t_[:], bb_d[l][:])
                bb_sb.append(t_)
            xk = mlpw.tile([128, 5 * 128], HF, tag="xk")
            nc.scalar.dma_start(xk[:], xT_d[:])

            # trunk weights -> SBUF
            tW0 = mlpw.tile([1, HID], HF, tag="tw0")
            nc.gpsimd.dma_start(tW0[:], tW0_d[:])
            tTs = mlpw.tile([1, T], HF, tag="tts")
            nc.sync.dma_start(tTs[:], tT_d[:])
            tb_sb = []
            for l in range(4):
                t_ = mlpw.tile([128, 4], FP, tag=f"tb{l}")
                qs[(l + 1) % 3].dma_start(t_[:], tb_d[l][:])
                tb_sb.append(t_)
            tW_sb = [None]
            for l in (1, 2, 3):
                w = mlpw.tile([128, 4 * HID], HF, tag=f"tw{l}")
                qs[l % 3].dma_start(w[:], tW_d[l][:])
                tW_sb.append(w)

            # GRU weights queued behind all MLP weights
            load_gru_weights()

            cur = xk
            for l in range(4):
                nk = 5 if l == 0 else 4
                func = AF.Relu if l < 3 else AF.Identity
                dst = branchT if l == 3 else mlps.tile([128, HID], HF, tag="mlpact")
                cur = mlp_layer(cur, nk, bW_sb[l], bb_sb[l], func, dst)

            # trunk layer 0 (K=1): out[t, fo] = tTs.T @ tW0
            ps = mlpp.tile([128, HID], FP, tag="mlpps")
            mm(ps[:], tTs[:], tW0[:], start=True, stop=True)
            sb = mlps.tile([128, HID], HF, tag="mlpsb")
            nc.vector.tensor_copy(sb[:], ps[:])
            pt = mlpp.tile([128, HID], HF, tag="mlppt")
            for m in range(4):
                nc.tensor.transpose(pt[:, m * 128:(m + 1) * 128],
                                    sb[:, m * 128:(m + 1) * 128], ident16[:])
            tact = mlps.tile([128, HID], HF, tag="mlpact")
            for m in range(4):
                nc.scalar.activation(tact[:, m * 128:(m + 1) * 128],
                                     pt[:, m * 128:(m + 1) * 128], AF.Relu,
                                     bias=tb_sb[0][:, m:m + 1])
            cur = tact
            for l in (1, 2, 3):
                dst = trunkT if l == 3 else mlps.tile([128, HID], HF, tag="mlpact")
                cur = mlp_layer(cur, 4, tW_sb[l], tb_sb[l], AF.Relu, dst)

            # seq[b,t] = sum_f branchT[f,b] * trunkT[f,t]  -> [B, T] psum
            ps_seq = mlpp.tile([128, 128], FP, tag="mlppsum")
            for k in range(4):
                mm(ps_seq[:], branchT[:, k * 128:(k + 1) * 128],
                                 trunkT[:, k * 128:(k + 1) * 128],
                                 start=(k == 0), stop=(k == 3))
            nc.scalar.copy(seq16[:], ps_seq[:])
            ps_seqT = mlpp.tile([128, 128], HF, tag="mlppsumT")
            nc.tensor.transpose(ps_seqT[:], seq16[:], ident16[:])
            nc.scalar.copy(seqT_sb[:], ps_seqT[:])

        # ================= GRU phase =================
        saug = const.tile([2, T * BC], HF)
        # partition-collapse seqT (t-major rows) into row 0 of saug
        nc.gpsimd.dma_start(saug[0:1, :], seqT_sb[:])
        nc.gpsimd.dma_start(saug[1:2, :], ones16k_d[:])

        with tc.tile_pool(name="gp", bufs=1, space=bass.MemorySpace.PSUM) as gp, \
             tc.tile_pool(name="gpt", bufs=1, space=bass.MemorySpace.PSUM) as gpt, \
             tc.tile_pool(name="gs", bufs=2) as gs:

            Pfill = gp.tile([128, 512], FP, tag="Pfill")

            def filler(cols):
                done = 0
                while done < cols:
                    n = min(512, cols - done)
                    mm(Pfill[:, 0:n], ident16[:], whh0[:, 0:n],
                       start=True, stop=True)
                    done += n

            def l0_mms(t):
                st = saug[:, t * BC:(t + 1) * BC]
                Prz0 = gp.tile([128, 512], FP, tag="Prz0")
                mm(Prz0[:], st, aug0[:, 0:512], start=True, stop=False)
                mm(Prz0[:], h0T[:, 0:128], whh0[:, 0:512], start=False, stop=False)
                mm(Prz0[:], h0T[:, 128:256], whh0[:, 768:1280], start=False, stop=True)
                AB0 = gp.tile([128, 512], FP, tag="AB0")  # (B0n | A0n)
                mm(AB0[:, 0:256], st, aug0[:, 512:768], start=True, stop=False)
                mm(AB0[:, 0:256], h0T[:, 0:128], whh0[:, 512:768], start=False, stop=False)
                mm(AB0[:, 0:256], h0T[:, 128:256], whh0[:, 1280:1536], start=False, stop=True)
                mm(AB0[:, 256:512], st, aug0n[:], start=True, stop=True)
                return Prz0, AB0

            def l1_mms():
                # L1 for step t-1: h0T/h1T hold step t-1 / t-2 states here
                Prz1 = gp.tile([128, 512], FP, tag="Prz1")
                mm(Prz1[:], ones1[:], aug1[:, 0:512], start=True, stop=False)
                mm(Prz1[:], h1T[:, 0:128], whh1[:, 0:512], start=False, stop=False)
                mm(Prz1[:], h1T[:, 128:256], whh1[:, 768:1280], start=False, stop=False)
                mm(Prz1[:], h0T[:, 0:128], wih1[:, 0:512], start=False, stop=False)
                mm(Prz1[:], h0T[:, 128:256], wih1[:, 768:1280], start=False, stop=True)
                AB1 = gp.tile([128, 512], FP, tag="AB1")  # (B1n | A1n)
                mm(AB1[:, 0:256], ones1[:], aug1[:, 512:768], start=True, stop=False)
                mm(AB1[:, 0:256], h1T[:, 0:128], whh1[:, 512:768], start=False, stop=False)
                mm(AB1[:, 0:256], h1T[:, 128:256], whh1[:, 1280:1536], start=False, stop=True)
                mm(AB1[:, 256:512], ones1[:], aug1n[:], start=True, stop=False)
                mm(AB1[:, 256:512], h0T[:, 0:128], wih1[:, 512:768], start=False, stop=False)
                mm(AB1[:, 256:512], h0T[:, 128:256], wih1[:, 1280:1536], start=False, stop=True)
                return Prz1, AB1

            def l0_chain(Prz0, AB0):
                rz0 = gs.tile([128, 512], HF, tag="rz0")
                nc.scalar.activation(rz0[:], Prz0[:], AF.Sigmoid)
                zh0 = gs.tile([128, 256], HF, tag="zh0")
                nc.gpsimd.tensor_mul(zh0[:], rz0[:, 256:512], h0[:])
                t1 = gs.tile([128, 256], HF, tag="t1")
                nc.vector.tensor_mul(t1[:], rz0[:, 0:256], AB0[:, 0:256])
                t2 = gs.tile([128, 256], HF, tag="t2")
                nc.vector.tensor_add(t2[:], t1[:], AB0[:, 256:512])
                n0 = gs.tile([128, 256], HF, tag="n0")
                nc.scalar.activation(n0[:], t2[:], AF.Tanh)
                t0 = gs.tile([128, 256], HF, tag="t0")
                nc.vector.scalar_tensor_tensor(
                    t0[:], rz0[:, 256:512], 1.0, n0[:], ALU.subtract, ALU.mult)
                # h = z*h - (z-1)*n = z*h + (1-z)*n
                nc.vector.tensor_sub(h0[:], zh0[:], t0[:])

            def l1_chain(Prz1, AB1):
                rz1 = gs.tile([128, 512], HF, tag="rz1")
                nc.scalar.activation(rz1[:], Prz1[:], AF.Sigmoid)
                zh1 = gs.tile([128, 256], HF, tag="zh1")
                nc.gpsimd.tensor_mul(zh1[:], rz1[:, 256:512], h1[:])
                t1b = gs.tile([128, 256], HF, tag="t1b")
                nc.vector.tensor_mul(t1b[:], rz1[:, 0:256], AB1[:, 0:256])
                t2b = gs.tile([128, 256], HF, tag="t2b")
                nc.vector.tensor_add(t2b[:], t1b[:], AB1[:, 256:512])
                n1 = gs.tile([128, 256], HF, tag="n1")
                nc.scalar.activation(n1[:], t2b[:], AF.Tanh)
                t0b = gs.tile([128, 256], HF, tag="t0b")
                nc.vector.scalar_tensor_tensor(
                    t0b[:], rz1[:, 256:512], 1.0, n1[:], ALU.subtract, ALU.mult)
                nc.gpsimd.tensor_sub(h1[:], zh1[:], t0b[:])

            def trs0():
                Ptr0 = gpt.tile([128, 256], HF, tag="Ptr0")
                nc.tensor.transpose(Ptr0[:, 0:128], h0[:, 0:128], ident16[:])
                nc.tensor.transpose(Ptr0[:, 128:256], h0[:, 128:256], ident16[:])
                nc.scalar.copy(h0T[:], Ptr0[:])

            def trs1():
                Ptr1 = gpt.tile([128, 256], HF, tag="Ptr1")
                nc.tensor.transpose(Ptr1[:, 0:128], h1[:, 0:128], ident16[:])
                nc.tensor.transpose(Ptr1[:, 128:256], h1[:, 128:256], ident16[:])
                nc.vector.tensor_copy(h1T[:], Ptr1[:])

            # L1 runs one step behind L0.
            pend = False
            for t in range(n_steps):
                Prz0, AB0 = l0_mms(t)
                if pend:
                    Prz1, AB1 = l1_mms()
                l0_chain(Prz0, AB0)
                if pend:
                    filler(FILL1)
                trs0()
                if pend:
                    l1_chain(Prz1, AB1)
                    filler(FILL2)
                    trs1()
                    filler(FILL3)
                pend = True
            # flush: L1 for the last step
            Prz1, AB1 = l1_mms()
            l1_chain(Prz1, AB1)
            trs1()

            # ---- projection ----
            Pout = gp.tile([128, 512], FP, tag="Prz1")
            mm(Pout[:, 0:NS], h1T[:, 0:128], pW[:, 0:NS], start=True, stop=False)
            mm(Pout[:, 0:NS], h1T[:, 128:256], pW[:, NS:2 * NS], start=False, stop=False)
            mm(Pout[:, 0:NS], ones1[:], pb[:], start=False, stop=True)
            out_sb = gs.tile([128, NS], FP, tag="outsb")
            nc.scalar.copy(out_sb[:], Pout[:, 0:NS])
            nc.gpsimd.dma_start(out_d[:], out_sb[:])

    _split_multi_waits(nc)
    return nc


def prep_inputs(inputs):
    """Host-side shared (per-core-identical) tensor prep."""
    f = np.float32
    hf = np.float16
    g = {}
    bWf = np.asarray(inputs['branch_Wf'], f)      # (512, 528)
    bWr = np.asarray(inputs['branch_Wr'], f)      # (3, 512, 512)
    w = np.zeros((NBP, HID), f)
    w[:NB] = bWf.T
    g['bW0'] = w.reshape(5, 128, HID).transpose(1, 0, 2).reshape(128, 5 * HID).astype(hf)
    for i in range(3):
        g[f'bW{i + 1}'] = np.ascontiguousarray(bWr[i].T).reshape(4, 128, HID).transpose(1, 0, 2).reshape(128, 4 * HID).astype(hf)
    g['bb0'] = np.asarray(inputs['branch_bf'], f).reshape(4, 128).T.copy()
    for i in range(3):
        g[f'bb{i + 1}'] = np.asarray(inputs['branch_br'][i], f).reshape(4, 128).T.copy()
    g['tW0'] = np.asarray(inputs['trunk_Wf'], f).T.astype(hf)          # (1, 512)
    tWr = np.asarray(inputs['trunk_Wr'], f)
    for i in range(3):
        g[f'tW{i + 1}'] = np.ascontiguousarray(tWr[i].T).reshape(4, 128, HID).transpose(1, 0, 2).reshape(128, 4 * HID).astype(hf)
    g['tb0'] = np.asarray(inputs['trunk_bf'], f).reshape(4, 128).T.copy()
    for i in range(3):
        g[f'tb{i + 1}'] = np.asarray(inputs['trunk_br'][i], f).reshape(4, 128).T.copy()
    g['tT'] = np.arange(T, dtype=f).reshape(1, T).astype(hf)

    def ktile2(W):
        return np.ascontiguousarray(np.asarray(W, f).T).reshape(2, 128, 768).transpose(1, 0, 2).reshape(128, 1536)
    g['whh0'] = ktile2(inputs['gru_Whh0']).astype(hf)
    g['whh1'] = ktile2(inputs['gru_Whh1']).astype(hf)
    g['wih1'] = ktile2(inputs['gru_Wih1']).astype(hf)
    don = float(np.asarray(inputs['don_bias'], f).reshape(-1)[0])
    w0 = np.asarray(inputs['gru_Wih0'], f)[:, 0]  # (768,)
    bih0 = np.asarray(inputs['gru_bih0'], f)
    bhh0 = np.asarray(inputs['gru_bhh0'], f)
    aug0 = np.zeros((2, 768), f)
    aug0[0, :512] = w0[:512]
    aug0[1, :512] = bih0[:512] + bhh0[:512] + don * w0[:512]
    aug0[1, 512:768] = bhh0[512:768]
    g['aug0'] = aug0.astype(hf)
    aug0n = np.zeros((2, 256), f)
    aug0n[0] = w0[512:768]
    aug0n[1] = bih0[512:768] + don * w0[512:768]
    g['aug0n'] = aug0n.astype(hf)
    bih1 = np.asarray(inputs['gru_bih1'], f)
    bhh1 = np.asarray(inputs['gru_bhh1'], f)
    aug1 = np.zeros((1, 768), f)
    aug1[0, :512] = bih1[:512] + bhh1[:512]
    aug1[0, 512:768] = bhh1[512:768]
    g['aug1'] = aug1.astype(hf)
    g['aug1n'] = bih1[512:768].reshape(1, 256).astype(hf)
    g['pW'] = np.ascontiguousarray(np.asarray(inputs['proj_W'], f).T).reshape(2, 128, NS).transpose(1, 0, 2).reshape(128, 2 * NS).astype(hf)
    g['pb'] = np.asarray(inputs['proj_b'], f).reshape(1, NS).astype(hf)
    g['ident16'] = np.eye(128, dtype=hf)
    g['zer'] = np.zeros((128, GH), hf)
    g['ones16k'] = np.ones((1, T * BC), hf)
    return g


def run(inputs, **spmd_kwargs):
    from concourse.bass_utils import run_bass_kernel_spmd

    if 'nc' not in _CACHE:
        _CACHE['nc'] = build_nc(T)
    nc = _CACHE['nc']

    shared = prep_inputs(inputs)
    x = np.asarray(inputs['x'], np.float32)
    in_maps = []
    for c in range(NCORES):
        xs = x[c * BC:(c + 1) * BC]          # (128, 528)
        xt = np.zeros((NBP, BC), np.float32)
        xt[:NB] = xs.T
        m = dict(shared)
        m['xT'] = xt.reshape(5, 128, BC).transpose(1, 0, 2).reshape(128, 5 * BC).astype(np.float16)
        in_maps.append(m)

    res = run_bass_kernel_spmd(nc, in_maps, list(range(NCORES)), **spmd_kwargs)
    out = np.concatenate([res.results[c]["out"] for c in range(NCORES)], axis=0)
    return out.astype(np.float32), res


def kernel(**inputs):
    out, _ = run(inputs)
    return out


if __name__ == "__main__":
    rng = np.random.RandomState(0)
    print("building nc...")
    nc = build_nc(2)
    print("built OK")
